# revision 1
# baseline (speedup 1.0000x reference)
import math
import sys

for _p in ("/root/.axon_site", "/root/.axon_site/_ro/trn_rl_repo", "/opt/trn_rl_repo"):
    if _p not in sys.path:
        sys.path.append(_p)

import numpy as np
import ml_dtypes

BF16 = ml_dtypes.bfloat16

B, L, NS = 8, 1024, 512
D, NH, DN = 512, 8, 256
E, K, HE = 8, 2, 2048
HD = D // NH
EPS = 1e-5
NCORES = 8

_NC = None


def _build():
    global _NC
    if _NC is not None:
        return _NC
    from concourse import bass, tile, mybir, masks

    f32 = mybir.dt.float32
    bf16 = mybir.dt.bfloat16
    AF = mybir.ActivationFunctionType
    OP = mybir.AluOpType

    nc = bass.Bass()
    x_h = nc.declare_dram_parameter("x", [L, D], f32, isOutput=False)
    scene_h = nc.declare_dram_parameter("scene", [NS, D], f32, isOutput=False)
    wattn_h = nc.declare_dram_parameter("wattn", [8, D, D], bf16, isOutput=False)
    battn_h = nc.declare_dram_parameter("battn", [8, D], f32, isOutput=False)
    wi_h = nc.declare_dram_parameter("wi", [64, 128, 512], bf16, isOutput=False)
    bi_h = nc.declare_dram_parameter("bi_t", [64, 128], f32, isOutput=False)
    wo_h = nc.declare_dram_parameter("wo", [32, 128, 512], bf16, isOutput=False)
    bo_h = nc.declare_dram_parameter("bo_t", [4, 128], f32, isOutput=False)
    out_h = nc.declare_dram_parameter("out", [L, D], f32, isOutput=True)

    with tile.TileContext(nc) as tc, \
         tc.tile_pool(name="sing", bufs=1) as sing, \
         tc.tile_pool(name="p_rm", bufs=2) as p_rm, \
         tc.tile_pool(name="p_resid", bufs=2) as p_resid, \
         tc.tile_pool(name="p_xn", bufs=2) as p_xn, \
         tc.tile_pool(name="p_xb", bufs=2) as p_xb, \
         tc.tile_pool(name="p_qkv", bufs=1) as p_qkv, \
         tc.tile_pool(name="p_o", bufs=8) as p_o, \
         tc.tile_pool(name="p_eT", bufs=4) as p_eT, \
         tc.tile_pool(name="p_ms", bufs=10) as p_ms, \
         tc.tile_pool(name="p_wi", bufs=4) as p_wi, \
         tc.tile_pool(name="p_wo", bufs=3) as p_wo, \
         tc.tile_pool(name="p_hid", bufs=3) as p_hid, \
         tc.tile_pool(name="p_orm", bufs=2) as p_orm, \
         tc.tile_pool(name="pA", bufs=2, space="PSUM") as pA, \
         tc.tile_pool(name="pB", bufs=4, space="PSUM") as pB, \
         tc.tile_pool(name="pT", bufs=2, space="PSUM") as pT:

        dma = nc.sync.dma_start

        ident = sing.tile([128, 128], f32, name="ident", tag="ident")
        masks.make_identity(nc, ident[:])
        ones_bf = sing.tile([128, 1], bf16, name="ones_bf", tag="ones")
        nc.vector.memset(ones_bf[:], 1.0)
        ones_f = sing.tile([128, 128], f32, name="ones_f", tag="ones_f")
        nc.vector.memset(ones_f[:], 1.0)

        w_attn = sing.tile([128, 8, 4, 512], bf16, name="w_attn", tag="w_attn")
        dma(out=w_attn[:], in_=wattn_h.rearrange("i (t p) d -> p i t d", p=128))
        b_attn = sing.tile([128, 8, 4], f32, name="b_attn", tag="b_attn")
        dma(out=b_attn[:], in_=battn_h.rearrange("i (t p) -> p i t", p=128))
        bi_sb = sing.tile([128, 64], f32, name="bi_sb", tag="bi_sb")
        dma(out=bi_sb[:], in_=bi_h.rearrange("b p -> p b"))
        bo_sb = sing.tile([128, 4], f32, name="bo_sb", tag="bo_sb")
        dma(out=bo_sb[:], in_=bo_h.rearrange("t p -> p t"))

        scene_T = sing.tile([128, 4, 512], bf16, name="scene_T", tag="scene_T")

        # scene -> feature-major bf16
        scene_rm = p_rm.tile([128, 4, 512], f32, name="scene_rm", tag="rm")
        dma(out=scene_rm[:], in_=scene_h.rearrange("(t p) d -> p t d", p=128))
        # PE transposes can carry only one sync wait; warm-up matmuls absorb
        # one producer wait so each transpose needs at most one.
        warm = pT.tile([128, 128], f32, name="warm", tag="tp")
        nc.tensor.matmul(warm[0:1, 0:1], ident[0:1, 0:1], ident[0:1, 0:1],
                         start=True, stop=True)
        for tt in range(4):
            for mt in range(4):
                tp = pT.tile([128, 128], f32, name="tp", tag="tp")
                nc.tensor.transpose(tp[:], scene_rm[:, tt, mt * 128:(mt + 1) * 128], ident[:])
                nc.scalar.activation(scene_T[:, mt, tt * 128:(tt + 1) * 128], tp[:], AF.Copy)

        # x -> feature-major f32 spine
        x_r = x_h.rearrange("(t p) d -> p t d", p=128)
        X_T = p_resid.tile([128, 4, 1024], f32, name="X_T", tag="resid")
        for half in range(2):
            x_rm = p_rm.tile([128, 4, 512], f32, name="x_rm", tag="rm")
            dma(out=x_rm[:], in_=x_r[:, half * 4:(half + 1) * 4, :])
            warmx = pA.tile([1, 1], f32, name="warmx", tag="pa")
            nc.tensor.matmul(warmx[:], x_rm[0:1, 0, 0:1], x_rm[0:1, 0, 0:1],
                             start=True, stop=True)
            for tt in range(4):
                q0 = (half * 4 + tt) * 128
                for mt in range(4):
                    tp = pT.tile([128, 128], f32, name="tp", tag="tp")
                    nc.tensor.transpose(tp[:], x_rm[:, tt, mt * 128:(mt + 1) * 128], ident[:])
                    nc.scalar.activation(X_T[:, mt, q0:q0 + 128], tp[:], AF.Copy)

        def layer_norm(src, xn):
            for qc in range(2):
                qs = slice(qc * 512, (qc + 1) * 512)
                xb = p_xb.tile([128, 4, 512], bf16, name="xb", tag="xbsq")
                sq = p_xb.tile([128, 4, 512], bf16, name="sq", tag="xbsq")
                nc.scalar.activation(xb[:], src[:, :, qs], AF.Copy)
                nc.scalar.activation(sq[:], src[:, :, qs], AF.Square)
                sum_ps = pA.tile([1, 512], f32, name="sum_ps", tag="pa")
                for kt in range(4):
                    nc.tensor.matmul(sum_ps[:], ones_bf[:], xb[:, kt, :],
                                     start=(kt == 0), stop=(kt == 3))
                sq_ps = pA.tile([1, 512], f32, name="sq_ps", tag="pa")
                for kt in range(4):
                    nc.tensor.matmul(sq_ps[:], ones_bf[:], sq[:, kt, :],
                                     start=(kt == 0), stop=(kt == 3))
                m = p_ms.tile([1, 512], f32, name="m", tag="ms")
                e2 = p_ms.tile([1, 512], f32, name="e2", tag="ms")
                mm = p_ms.tile([1, 512], f32, name="mm", tag="ms")
                var = p_ms.tile([1, 512], f32, name="var", tag="ms")
                sd = p_ms.tile([1, 512], f32, name="sd", tag="ms")
                r = p_ms.tile([1, 512], f32, name="r", tag="ms")
                nc.vector.tensor_scalar_mul(m[:], sum_ps[:], 1.0 / 512.0)
                nc.vector.tensor_scalar(e2[:], sq_ps[:], 1.0 / 512.0, EPS, OP.mult, OP.add)
                nc.vector.tensor_tensor(mm[:], m[:], m[:], OP.mult)
                nc.vector.tensor_tensor(var[:], e2[:], mm[:], OP.subtract)
                nc.scalar.activation(sd[:], var[:], AF.Sqrt)
                nc.vector.reciprocal(r[:], sd[:])
                rb_ps = pA.tile([128, 512], f32, name="rb_ps", tag="pa")
                nc.tensor.matmul(rb_ps[:], ones_f[0:1, :], r[:], start=True, stop=True)
                mb_ps = pA.tile([128, 512], f32, name="mb_ps", tag="pa")
                nc.tensor.matmul(mb_ps[:], ones_f[0:1, :], m[:], start=True, stop=True)
                for mt in range(4):
                    xs = p_ms.tile([128, 512], f32, name="xs", tag="ms")
                    nc.vector.tensor_tensor(xs[:], src[:, mt, qs], mb_ps[:], OP.subtract)
                    nc.vector.tensor_tensor(xn[:, mt, qs], xs[:], rb_ps[:], OP.mult)

        def attention(widx, xq_T, kv_T, kv_len, resid_in, resid_out):
            nkp = kv_len // 128
            nkc = kv_len // 512
            Q_T = p_qkv.tile([128, 4, 1024], bf16, name="Q_T", tag="q")
            for mt in range(4):
                for qc in range(2):
                    qs = slice(qc * 512, (qc + 1) * 512)
                    q_ps = pA.tile([128, 512], f32, name="q_ps", tag="pa")
                    for kt in range(4):
                        nc.tensor.matmul(q_ps[:], w_attn[:, widx, kt, mt * 128:(mt + 1) * 128],
                                         xq_T[:, kt, qs], start=(kt == 0), stop=(kt == 3))
                    nc.scalar.activation(Q_T[:, mt, qs], q_ps[:], AF.Identity,
                                         bias=b_attn[:, widx, mt:mt + 1])
            K_T = p_qkv.tile([128, 4, 1024], bf16, name="K_T", tag="k")
            for mt in range(4):
                for kc in range(nkc):
                    ks = slice(kc * 512, (kc + 1) * 512)
                    k_ps = pA.tile([128, 512], f32, name="k_ps", tag="pa")
                    for kt in range(4):
                        nc.tensor.matmul(k_ps[:], w_attn[:, widx + 1, kt, mt * 128:(mt + 1) * 128],
                                         kv_T[:, kt, ks], start=(kt == 0), stop=(kt == 3))
                    nc.scalar.activation(K_T[:, mt, ks], k_ps[:], AF.Identity,
                                         bias=b_attn[:, widx + 1, mt:mt + 1])
            # V weights are column-permuted on host: dout' 0..255 = heads 0,2,4,6;
            # dout' 256..511 = heads 1,3,5,7. Even heads -> psum rows 0..63 (denom 64),
            # odd heads -> psum rows 64..127 (denom 0), so head pairs share one
            # [128,512] Oh tile and o-proj contracts K=128 at base partition 0.
            V_ev = p_qkv.tile([128, 8, 4, 65], bf16, name="V_ev", tag="ve")
            V_od = p_qkv.tile([128, 8, 4, 128], bf16, name="V_od", tag="vo")
            nc.vector.memset(V_ev[:, 0:nkp, :, 64:65], 1.0)
            nc.vector.memset(V_od[:, 0:nkp, :, 0:1], 1.0)
            nc.vector.memset(V_od[:, 0:nkp, :, 1:64], 0.0)
            for kp in range(nkp):
                v_ps = pA.tile([128, 2, 4, 64], f32, name="v_ps", tag="pa")
                for kt in range(4):
                    nc.tensor.matmul(v_ps[:], kv_T[:, kt, kp * 128:(kp + 1) * 128],
                                     w_attn[:, widx + 2, kt, :], start=(kt == 0), stop=(kt == 3))
                nc.scalar.activation(V_ev[:, kp, :, 0:64], v_ps[:, 0, :, :], AF.Copy)
                nc.scalar.activation(V_od[:, kp, :, 64:128], v_ps[:, 1, :, :], AF.Copy)
            for qc in range(2):
                qs = slice(qc * 512, (qc + 1) * 512)
                Ohs = []
                for hb in range(4):
                    OhP = p_o.tile([128, 512], bf16, name="OhP", tag="oh")
                    o_pss = []
                    for par in range(2):
                        po = par * 64
                        o_ps = pB.tile([128, 512], f32, name="o_ps", tag="pb")
                        for kp in range(nkp):
                            s_ps = pA.tile([128, 512], f32, name="s_ps", tag="pa")
                            nc.tensor.matmul(s_ps[:], K_T[po:po + 64, hb, kp * 128:(kp + 1) * 128],
                                             Q_T[po:po + 64, hb, qs], start=True, stop=True)
                            e_t = p_eT.tile([128, 512], bf16, name="e_t", tag="et")
                            nc.scalar.activation(e_t[:], s_ps[:], AF.Exp, scale=0.125)
                            if par == 0:
                                nc.tensor.matmul(o_ps[0:65, :], V_ev[:, kp, hb, :], e_t[:],
                                                 start=(kp == 0), stop=(kp == nkp - 1))
                            else:
                                nc.tensor.matmul(o_ps[:], V_od[:, kp, hb, :], e_t[:],
                                                 start=(kp == 0), stop=(kp == nkp - 1))
                        o_pss.append(o_ps)
                    rc = p_ms.tile([65, 512], f32, name="rc", tag="ms")
                    nc.vector.reciprocal(rc[64:65, :], o_pss[0][64:65, :])
                    nc.vector.reciprocal(rc[0:1, :], o_pss[1][0:1, :])
                    rb_ps = pB.tile([128, 512], f32, name="rb_ps", tag="pb")
                    nc.tensor.matmul(rb_ps[0:64, :], ones_f[64:65, 0:64], rc[64:65, :],
                                     start=True, stop=True)
                    nc.tensor.matmul(rb_ps[64:128, :], ones_f[0:1, 0:64], rc[0:1, :],
                                     start=True, stop=True)
                    rb_sb = p_eT.tile([128, 512], bf16, name="rb_sb", tag="et")
                    nc.scalar.activation(rb_sb[:], rb_ps[:], AF.Copy)
                    nc.vector.tensor_tensor(OhP[0:64, :], o_pss[0][0:64, :], rb_sb[0:64, :], OP.mult)
                    nc.vector.tensor_tensor(OhP[64:128, :], o_pss[1][64:128, :], rb_sb[64:128, :], OP.mult)
                    Ohs.append(OhP)
                for mt in range(4):
                    ps = pA.tile([128, 512], f32, name="ps_op", tag="pa")
                    for hb in range(4):
                        nc.tensor.matmul(ps[:], w_attn[:, widx + 3, hb, mt * 128:(mt + 1) * 128],
                                         Ohs[hb][:], start=(hb == 0), stop=(hb == 3))
                    nc.vector.scalar_tensor_tensor(resid_out[:, mt, qs], ps[:],
                                                   b_attn[:, widx + 3, mt:mt + 1],
                                                   resid_in[:, mt, qs], OP.add, OP.add)

        def moe(xn3, resid_in, resid_out):
            for qc in range(2):
                qs = slice(qc * 512, (qc + 1) * 512)
                eos = [pB.tile([128, 512], f32, name=f"eo{mt}", tag="pb") for mt in range(4)]
                for e in range(2):
                    for j in range(16):
                        bv_i = e * 32 + j
                        bg_i = e * 32 + 16 + j
                        wo_i = e * 16 + j
                        wv_t = p_wi.tile([128, 512], bf16, name="wv_t", tag="wi")
                        dma(out=wv_t[:], in_=wi_h[bv_i, :, :])
                        wg_t = p_wi.tile([128, 512], bf16, name="wg_t", tag="wi")
                        dma(out=wg_t[:], in_=wi_h[bg_i, :, :])
                        wo_t = p_wo.tile([128, 512], bf16, name="wo_t", tag="wo")
                        dma(out=wo_t[:], in_=wo_h[wo_i, :, :])
                        gate_ps = pA.tile([128, 512], f32, name="gate_ps", tag="pa")
                        for kt in range(4):
                            nc.tensor.matmul(gate_ps[:], wg_t[:, kt * 128:(kt + 1) * 128],
                                             xn3[:, kt, qs], start=(kt == 0), stop=(kt == 3))
                        sg = p_eT.tile([128, 512], bf16, name="sg", tag="et")
                        nc.scalar.activation(sg[:], gate_ps[:], AF.Silu, bias=bi_sb[:, bg_i:bg_i + 1])
                        val_ps = pA.tile([128, 512], f32, name="val_ps", tag="pa")
                        for kt in range(4):
                            nc.tensor.matmul(val_ps[:], wv_t[:, kt * 128:(kt + 1) * 128],
                                             xn3[:, kt, qs], start=(kt == 0), stop=(kt == 3))
                        hid = p_hid.tile([128, 512], bf16, name="hid", tag="hid")
                        nc.vector.scalar_tensor_tensor(hid[:], val_ps[:], bi_sb[:, bv_i:bv_i + 1],
                                                       sg[:], OP.add, OP.mult)
                        first = (e == 0 and j == 0)
                        last = (e == 1 and j == 15)
                        for mt in range(4):
                            nc.tensor.matmul(eos[mt][:], wo_t[:, mt * 128:(mt + 1) * 128],
                                             hid[:], start=first, stop=last)
                for mt in range(4):
                    nc.vector.scalar_tensor_tensor(resid_out[:, mt, qs], eos[mt][:],
                                                   bo_sb[:, mt:mt + 1],
                                                   resid_in[:, mt, qs], OP.add, OP.add)

        xn1 = p_xn.tile([128, 4, 1024], bf16, name="xn1", tag="xn")
        layer_norm(X_T, xn1)
        X2 = p_resid.tile([128, 4, 1024], f32, name="X2", tag="resid")
        attention(0, xn1, scene_T, 512, X_T, X2)
        xn2 = p_xn.tile([128, 4, 1024], bf16, name="xn2", tag="xn")
        layer_norm(X2, xn2)
        X3 = p_resid.tile([128, 4, 1024], f32, name="X3", tag="resid")
        attention(4, xn2, xn2, 1024, X2, X3)
        xn3 = p_xn.tile([128, 4, 1024], bf16, name="xn3", tag="xn")
        layer_norm(X3, xn3)
        OUT_T = p_resid.tile([128, 4, 1024], f32, name="OUT_T", tag="resid")
        moe(xn3, X3, OUT_T)

        out_r = out_h.rearrange("(t p) d -> p t d", p=128)
        warmo = pT.tile([128, 128], f32, name="warmo", tag="tp")
        nc.tensor.matmul(warmo[0:1, 0:1], OUT_T[0:1, 3, 1023:1024],
                         OUT_T[0:1, 3, 1023:1024], start=True, stop=True)
        for tq in range(8):
            orm = p_orm.tile([128, 512], f32, name="orm", tag="orm")
            for mt in range(4):
                tp = pT.tile([128, 128], f32, name="tp", tag="tp")
                nc.tensor.transpose(tp[:], OUT_T[:, mt, tq * 128:(tq + 1) * 128], ident[:])
                nc.scalar.activation(orm[:, mt * 128:(mt + 1) * 128], tp[:], AF.Copy)
            dma(out=out_r[:, tq, :], in_=orm[:])

    _legalize_waits(nc)
    _NC = nc
    return nc


def _legalize_waits(nc):
    # DMACopy/Matmult/Ldweights hardware encodings hold a single sem wait
    # (transpose matmuls and direct2d DMAs have one EVENTS slot); move the
    # extras onto standalone EventSemaphore instructions issued just before
    # on the same engine queue.
    from concourse import mybir
    n = 0
    for fn in nc.m.functions:
        for blk in fn.blocks:
            out = []
            for inst in blk.instructions:
                si = getattr(inst, "sync_info", None)
                ow = list(si.on_wait) if si is not None else []
                if len(ow) > 1 and getattr(inst, "opcode", None) is not None:
                    for j, w in enumerate(ow[:-1]):
                        out.append(mybir.InstEventSemaphore(
                            name=f"{inst.name}-wx{j}",
                            engine=inst.engine,
                            sync_info=mybir.SyncInfo(on_wait=[w], on_update=[]),
                        ))
                        n += 1
                    inst.sync_info = mybir.SyncInfo(
                        on_wait=[ow[-1]], on_update=list(si.on_update))
                out.append(inst)
            blk.instructions = out
    return n


def _silu(v):
    return v / (1.0 + np.exp(-v))


def _softmax(v):
    m = v.max(axis=-1, keepdims=True)
    ex = np.exp(v - m)
    return ex / ex.sum(axis=-1, keepdims=True)


def _prepare(inputs):
    inp = {k: np.asarray(v, dtype=np.float32) for k, v in inputs.items()}
    x = inp["x"]
    scene = inp["scene_tokens"]
    t = inp["t"]
    g = inp["scene_norm_g"]
    bvec = inp["scene_norm_b"]

    half = D // 2
    freqs = np.exp(-math.log(10000.0) * np.arange(half, dtype=np.float32) / (half - 1)).astype(np.float32)
    ang = t[:, None] * freqs[None, :]
    temb = np.concatenate([np.cos(ang), np.sin(ang)], axis=-1).astype(np.float32)
    ncv = _silu(temb @ inp["ne_w1"] + inp["ne_b1"]) @ inp["ne_w2"] + inp["ne_b2"]

    mod1 = ncv @ inp["ncsa_mod_w"] + inp["ncsa_mod_b"]
    shift1, scale1 = mod1[:, :D], mod1[:, D:]
    mod2 = ncv @ inp["moe_mod_w"] + inp["moe_mod_b"]
    shift2, scale2 = mod2[:, :D], mod2[:, D:]

    probs = _softmax(ncv @ inp["router_w"])
    ti = np.argsort(-probs, axis=-1, kind="stable")[:, :K]
    tw = np.take_along_axis(probs, ti, axis=-1)
    tw = tw / np.clip(tw.sum(-1, keepdims=True), 1e-8, None)

    ca_wq_e = g[:, None] * inp["ca_wq"]
    ca_bq_e = inp["ca_bq"] + bvec @ inp["ca_wq"]
    ca_bo_e = inp["ca_bo"] + inp["ca_bv"] @ inp["ca_wo"]
    zero = np.zeros(D, np.float32)
    # even heads first, then odd heads (see V layout note in _build)
    vperm = np.concatenate([np.arange(h * HD, (h + 1) * HD) for h in (0, 2, 4, 6, 1, 3, 5, 7)])

    in_maps = []
    for b in range(B):
        s1 = 1.0 + scale1[b]
        sa_wq_e = s1[:, None] * inp["sa_wq"]
        sa_bq_e = inp["sa_bq"] + shift1[b] @ inp["sa_wq"]
        sa_wk_e = s1[:, None] * inp["sa_wk"]
        sa_bk_e = inp["sa_bk"] + shift1[b] @ inp["sa_wk"]
        sa_wv_e = s1[:, None] * inp["sa_wv"]
        sa_bv_e = inp["sa_bv"] + shift1[b] @ inp["sa_wv"]
        sa_bo_e = inp["sa_bo"] + sa_bv_e @ inp["sa_wo"]

        wattn = np.stack([ca_wq_e, inp["ca_wk"], inp["ca_wv"][:, vperm], inp["ca_wo"],
                          sa_wq_e, sa_wk_e, sa_wv_e[:, vperm], inp["sa_wo"]]).astype(BF16)
        battn = np.stack([ca_bq_e, inp["ca_bk"], zero, ca_bo_e,
                          sa_bq_e, sa_bk_e, zero, sa_bo_e]).astype(np.float32)

        s2 = 1.0 + scale2[b]
        Wis, bis, Wos = [], [], []
        bo_moe = np.zeros(D, np.float32)
        for k in range(K):
            eidx = int(ti[b, k])
            w = np.float32(tw[b, k])
            Wi_e = inp["fc_in_w"][eidx]
            Wis.append(s2[:, None] * Wi_e)
            bis.append(inp["fc_in_b"][eidx] + shift2[b] @ Wi_e)
            Wos.append(w * inp["fc_out_w"][eidx])
            bo_moe = bo_moe + w * inp["fc_out_b"][eidx]
        Wi_cat = np.concatenate(Wis, axis=1)
        bi_cat = np.concatenate(bis, axis=0)
        Wo_cat = np.concatenate(Wos, axis=0)

        wi_pt = np.ascontiguousarray(
            Wi_cat.reshape(4, 128, 64, 128).transpose(2, 1, 0, 3).reshape(64, 128, 512)).astype(BF16)
        wo_pt = np.ascontiguousarray(Wo_cat.reshape(32, 128, 512)).astype(BF16)
        bi_pt = np.ascontiguousarray(bi_cat.reshape(64, 128)).astype(np.float32)
        bo_pt = np.ascontiguousarray(bo_moe.reshape(4, 128)).astype(np.float32)

        in_maps.append({
            "x": np.ascontiguousarray(x[b]),
            "scene": np.ascontiguousarray(scene[b]),
            "wattn": np.ascontiguousarray(wattn),
            "battn": np.ascontiguousarray(battn),
            "wi": wi_pt,
            "bi_t": bi_pt,
            "wo": wo_pt,
            "bo_t": bo_pt,
        })
    return in_maps


def _run(in_maps, trace=False):
    from concourse.bass_utils import run_bass_kernel_spmd
    nc = _build()
    return run_bass_kernel_spmd(nc, in_maps, list(range(NCORES)), trace=trace)


def kernel(**inputs):
    in_maps = _prepare(inputs)
    res = _run(in_maps)
    return np.stack([np.asarray(res.results[i]["out"], dtype=np.float32) for i in range(B)])



# revision 11
# speedup vs baseline: 1.7292x; 1.7292x over previous
import math
import sys

for _p in ("/root/.axon_site", "/root/.axon_site/_ro/trn_rl_repo", "/opt/trn_rl_repo"):
    if _p not in sys.path:
        sys.path.append(_p)

import numpy as np
import ml_dtypes

BF16 = ml_dtypes.bfloat16

B, L, NS = 8, 1024, 512
D, NH, DN = 512, 8, 256
E, K, HE = 8, 2, 2048
HD = D // NH
EPS = 1e-5
NCORES = 8

_NC = None


def _build():
    global _NC
    if _NC is not None:
        return _NC
    from concourse import bass, tile, mybir, masks

    f32 = mybir.dt.float32
    f32r = mybir.dt.float32r
    bf16 = mybir.dt.bfloat16
    AF = mybir.ActivationFunctionType
    OP = mybir.AluOpType

    nc = bass.Bass()
    x_h = nc.declare_dram_parameter("x", [L, D], f32, isOutput=False)
    scene_h = nc.declare_dram_parameter("scene", [NS, D], f32, isOutput=False)
    wattn_h = nc.declare_dram_parameter("wattn", [8, D, D], bf16, isOutput=False)
    battn_h = nc.declare_dram_parameter("battn", [8, D], f32, isOutput=False)
    wi_h = nc.declare_dram_parameter("wi", [64, 128, 512], bf16, isOutput=False)
    bi_h = nc.declare_dram_parameter("bi_t", [64, 128], f32, isOutput=False)
    wo_h = nc.declare_dram_parameter("wo", [32, 128, 512], bf16, isOutput=False)
    bo_h = nc.declare_dram_parameter("bo_t", [4, 128], f32, isOutput=False)
    out_h = nc.declare_dram_parameter("out", [L, D], f32, isOutput=True)

    with tile.TileContext(nc) as tc, \
         tc.tile_pool(name="sing", bufs=1) as sing, \
         tc.tile_pool(name="p_rm", bufs=3) as p_rm, \
         tc.tile_pool(name="p_resid", bufs=2) as p_resid, \
         tc.tile_pool(name="p_xn", bufs=2) as p_xn, \
         tc.tile_pool(name="p_qkv", bufs=1) as p_qkv, \
         tc.tile_pool(name="p_o", bufs=8) as p_o, \
         tc.tile_pool(name="p_eT", bufs=4) as p_eT, \
         tc.tile_pool(name="p_ms", bufs=12) as p_ms, \
         tc.tile_pool(name="p_xb", bufs=2) as p_xb, \
         tc.tile_pool(name="p_wi", bufs=4) as p_wi, \
         tc.tile_pool(name="p_wo", bufs=3) as p_wo, \
         tc.tile_pool(name="p_hid", bufs=3) as p_hid, \
         tc.tile_pool(name="p_orm", bufs=2) as p_orm, \
         tc.tile_pool(name="pA", bufs=2, space="PSUM") as pA, \
         tc.tile_pool(name="pOp", bufs=4, space="PSUM") as pOp, \
         tc.tile_pool(name="pB", bufs=2, space="PSUM") as pB:

        dma = nc.sync.dma_start

        ident = sing.tile([128, 128], f32, name="ident", tag="ident")
        masks.make_identity(nc, ident[:])
        ones_bf = sing.tile([128, 1], bf16, name="ones_bf", tag="ones")
        nc.vector.memset(ones_bf[:], 1.0)
        ones_row = sing.tile([1, 128], bf16, name="ones_row", tag="ones_r")
        nc.vector.memset(ones_row[:], 1.0)
        ones_sq = sing.tile([128, 64], bf16, name="ones_sq", tag="ones_sq")
        nc.vector.memset(ones_sq[:], 1.0)
        ones_rf = sing.tile([128, 1], f32, name="ones_rf", tag="ones_rf")
        nc.vector.memset(ones_rf[:], 1.0)

        # --- HAM warm-up: keep PE busy through the input DMA window so the
        # clock gate opens before real matmuls start; also preload the
        # ln/exp activation table set during the idle window.
        dummy = p_ms.tile([1, 1], f32, name="dummy", tag="ms")
        nc.scalar.activation(dummy[:], ident[0:1, 0:1], AF.Ln)
        dummy2 = p_ms.tile([1, 1], f32, name="dummy2", tag="ms")
        nc.scalar.activation(dummy2[:], ident[0:1, 0:1], AF.Exp)
        for _w in range(48):
            spin = pB.tile([128, 512], f32, name="spin", tag="pb")
            nc.tensor.matmul(spin[:, 0:128], ident[:], ident[:],
                             start=True, stop=True)

        # --- input DMAs, finest-first so transposes can start early
        x_r = x_h.rearrange("(t p) d -> p t d", p=128)
        x_rms = []
        for tq in range(8):
            x_rm = p_rm.tile([128, 512], f32, name=f"x_rm{tq}", tag="rm")
            dma(out=x_rm[:], in_=x_r[:, tq, :])
            x_rms.append(x_rm)
        scene_rm = sing.tile([128, 4, 512], f32, name="scene_rm", tag="scrm")
        dma(out=scene_rm[:], in_=scene_h.rearrange("(t p) d -> p t d", p=128))
        w_attn = sing.tile([128, 8, 4, 512], bf16, name="w_attn", tag="w_attn")
        dma(out=w_attn[:], in_=wattn_h.rearrange("i (t p) d -> p i t d", p=128))
        b_attn = sing.tile([128, 8, 4], f32, name="b_attn", tag="b_attn")
        dma(out=b_attn[:], in_=battn_h.rearrange("i (t p) -> p i t", p=128))
        bi_sb = sing.tile([128, 64], f32, name="bi_sb", tag="bi_sb")
        dma(out=bi_sb[:], in_=bi_h.rearrange("b p -> p b"))
        bo_sb = sing.tile([128, 4], f32, name="bo_sb", tag="bo_sb")
        dma(out=bo_sb[:], in_=bo_h.rearrange("t p -> p t"))

        # --- x -> feature-major f32 spine (PE transpose, f32r mode)
        X_T = p_resid.tile([128, 4, 1024], f32, name="X_T", tag="resid")
        for tq in range(8):
            for mt in range(4):
                tp = pB.tile([128, 512], f32, name="tp", tag="pb")
                nc.tensor.transpose(tp[:, 0:128],
                                    x_rms[tq][:, mt * 128:(mt + 1) * 128], ident[:])
                if mt % 2 == 0:
                    nc.scalar.activation(X_T[:, mt, tq * 128:(tq + 1) * 128],
                                         tp[:, 0:128], AF.Copy)
                else:
                    nc.vector.tensor_scalar_mul(X_T[:, mt, tq * 128:(tq + 1) * 128],
                                                tp[:, 0:128], 1.0)

        scene_T = sing.tile([128, 4, 512], bf16, name="scene_T", tag="scene_T")
        for tt in range(4):
            for mt in range(4):
                tp = pB.tile([128, 512], f32, name="tp", tag="pb")
                nc.tensor.transpose(tp[:, 0:128],
                                    scene_rm[:, tt, mt * 128:(mt + 1) * 128], ident[:])
                if mt % 2 == 0:
                    nc.scalar.activation(scene_T[:, mt, tt * 128:(tt + 1) * 128],
                                         tp[:, 0:128], AF.Copy)
                else:
                    nc.vector.tensor_scalar_mul(scene_T[:, mt, tt * 128:(tt + 1) * 128],
                                                tp[:, 0:128], 1.0)

        def layer_norm(src, xn):
            for qc in range(2):
                qs = slice(qc * 512, (qc + 1) * 512)
                xb = p_xb.tile([128, 4, 512], bf16, name="xb", tag="xbsq")
                nc.scalar.activation(xb[:], src[:, :, qs], AF.Copy)
                sq = p_xb.tile([128, 4, 512], bf16, name="sq", tag="xbsq")
                nc.scalar.activation(sq[:], src[:, :, qs], AF.Square)
                sum_ps = pA.tile([1, 512], f32, name="sum_ps", tag="pa")
                for kt in range(4):
                    nc.tensor.matmul(sum_ps[:], ones_bf[:], xb[:, kt, :],
                                     start=(kt == 0), stop=(kt == 3))
                sq_ps = pA.tile([1, 512], f32, name="sq_ps", tag="pa")
                for kt in range(4):
                    nc.tensor.matmul(sq_ps[:], ones_bf[:], sq[:, kt, :],
                                     start=(kt == 0), stop=(kt == 3))
                m_bf = p_ms.tile([1, 512], bf16, name="m_bf", tag="ms")
                nc.vector.tensor_scalar_mul(m_bf[:], sum_ps[:], 1.0 / 512.0)
                m = p_ms.tile([1, 512], f32, name="m", tag="ms")
                nc.vector.tensor_scalar_mul(m[:], sum_ps[:], 1.0 / 512.0)
                e2 = p_ms.tile([1, 512], f32, name="e2", tag="ms")
                nc.vector.tensor_scalar(e2[:], sq_ps[:], 1.0 / 512.0, EPS, OP.mult, OP.add)
                mm = p_ms.tile([1, 512], f32, name="mm", tag="ms")
                nc.vector.tensor_tensor(mm[:], m[:], m[:], OP.mult)
                var = p_ms.tile([1, 512], f32, name="var", tag="ms")
                nc.vector.tensor_tensor(var[:], e2[:], mm[:], OP.subtract)
                # 1/sqrt(var) = exp(-0.5*ln(var)) -- stays in the ln/exp
                # table set (no Sqrt table switch)
                lnv = p_ms.tile([1, 512], f32, name="lnv", tag="ms")
                nc.scalar.activation(lnv[:], var[:], AF.Ln)
                r_bf = p_ms.tile([1, 512], bf16, name="r_bf", tag="ms")
                nc.scalar.activation(r_bf[:], lnv[:], AF.Exp, scale=-0.5)
                rb_ps = pB.tile([128, 512], f32, name="rb_ps", tag="pb")
                nc.tensor.matmul(rb_ps[:], ones_row[:], r_bf[:], start=True, stop=True)
                mb_ps = pB.tile([128, 512], f32, name="mb_ps", tag="pb")
                nc.tensor.matmul(mb_ps[:], ones_row[:], m_bf[:], start=True, stop=True)
                for mt in range(4):
                    xs = p_ms.tile([128, 512], bf16, name="xs", tag="msx")
                    nc.vector.tensor_tensor(xs[:], src[:, mt, qs], mb_ps[:], OP.subtract)
                    nc.vector.tensor_tensor(xn[:, mt, qs], xs[:], rb_ps[:], OP.mult)

        def attention(widx, xq_T, kv_T, kv_len, resid_in, resid_out):
            nkp = kv_len // 128
            nkc = kv_len // 512
            Q_T = p_qkv.tile([128, 4, 1024], bf16, name="Q_T", tag="q")
            for mt in range(4):
                for qc in range(2):
                    qs = slice(qc * 512, (qc + 1) * 512)
                    q_ps = pA.tile([128, 512], f32, name="q_ps", tag="pa")
                    for kt in range(4):
                        nc.tensor.matmul(q_ps[:], w_attn[:, widx, kt, mt * 128:(mt + 1) * 128],
                                         xq_T[:, kt, qs], start=(kt == 0), stop=(kt == 3))
                    nc.vector.tensor_scalar_add(Q_T[:, mt, qs], q_ps[:],
                                                b_attn[:, widx, mt:mt + 1])
            K_T = p_qkv.tile([128, 4, 1024], bf16, name="K_T", tag="k")
            for mt in range(4):
                for kc in range(nkc):
                    ks = slice(kc * 512, (kc + 1) * 512)
                    k_ps = pA.tile([128, 512], f32, name="k_ps", tag="pa")
                    for kt in range(4):
                        nc.tensor.matmul(k_ps[:], w_attn[:, widx + 1, kt, mt * 128:(mt + 1) * 128],
                                         kv_T[:, kt, ks], start=(kt == 0), stop=(kt == 3))
                    nc.vector.tensor_scalar_add(K_T[:, mt, ks], k_ps[:],
                                                b_attn[:, widx + 1, mt:mt + 1])
            # V with packed softmax denominators (see baseline comment)
            V_ev = p_qkv.tile([128, 8, 4, 65], bf16, name="V_ev", tag="ve")
            V_od = p_qkv.tile([128, 8, 4, 128], bf16, name="V_od", tag="vo")
            nc.vector.memset(V_ev[:, 0:nkp, :, 64:65], 1.0)
            nc.vector.memset(V_od[:, 0:nkp, :, 0:1], 1.0)
            nc.vector.memset(V_od[:, 0:nkp, :, 1:64], 0.0)
            for kp in range(nkp):
                v_ps = pA.tile([128, 2, 4, 64], f32, name="v_ps", tag="pa")
                for kt in range(4):
                    nc.tensor.matmul(v_ps[:], kv_T[:, kt, kp * 128:(kp + 1) * 128],
                                     w_attn[:, widx + 2, kt, :], start=(kt == 0), stop=(kt == 3))
                nc.vector.tensor_scalar_mul(V_ev[:, kp, :, 0:64], v_ps[:, 0, :, :], 1.0)
                nc.vector.tensor_scalar_mul(V_od[:, kp, :, 64:128], v_ps[:, 1, :, :], 1.0)

            def emit_norm(st):
                o_pss, rcb, qs = st
                rb_ps = pB.tile([128, 512], f32, name="rb_ps", tag="pb")
                nc.tensor.matmul(rb_ps[0:64, :], ones_sq[64:65, 0:64], rcb[64:65, :],
                                 start=True, stop=True)
                nc.tensor.matmul(rb_ps[64:128, :], ones_sq[0:1, 0:64], rcb[0:1, :],
                                 start=True, stop=True)
                rb_sb = p_eT.tile([128, 512], bf16, name="rb_sb", tag="et")
                nc.vector.tensor_scalar_mul(rb_sb[:], rb_ps[:], 1.0)
                OhP = p_o.tile([128, 512], bf16, name="OhP", tag="oh")
                nc.vector.tensor_tensor(OhP[0:64, :], o_pss[0][0:64, :], rb_sb[0:64, :], OP.mult)
                nc.vector.tensor_tensor(OhP[64:128, :], o_pss[1][64:128, :], rb_sb[64:128, :], OP.mult)
                return OhP

            for qc in range(2):
                qs = slice(qc * 512, (qc + 1) * 512)
                pend = None
                Ohs = []
                for hb in range(4):
                    o_pss = []
                    for par in range(2):
                        po = par * 64
                        o_ps = pOp.tile([128, 512], f32, name="o_ps", tag="op")
                        for kp in range(nkp):
                            s_ps = pA.tile([128, 512], f32, name="s_ps", tag="pa")
                            nc.tensor.matmul(s_ps[:], K_T[po:po + 64, hb, kp * 128:(kp + 1) * 128],
                                             Q_T[po:po + 64, hb, qs], start=True, stop=True)
                            e_t = p_eT.tile([128, 512], bf16, name="e_t", tag="et")
                            nc.scalar.activation(e_t[:], s_ps[:], AF.Exp, scale=0.125)
                            if par == 0:
                                nc.tensor.matmul(o_ps[0:65, :], V_ev[:, kp, hb, :], e_t[:],
                                                 start=(kp == 0), stop=(kp == nkp - 1))
                            else:
                                nc.tensor.matmul(o_ps[:], V_od[:, kp, hb, :], e_t[:],
                                                 start=(kp == 0), stop=(kp == nkp - 1))
                        o_pss.append(o_ps)
                    # softmax denominators: 1/d = exp(-ln d) on ScalarE --
                    # same table set as the attention exps, ~5x faster than
                    # DVE reciprocal; consumed one hb later (PE never waits)
                    lnd = p_ms.tile([65, 512], f32, name="lnd", tag="ms")
                    nc.scalar.activation(lnd[64:65, :], o_pss[0][64:65, :], AF.Ln)
                    nc.scalar.activation(lnd[0:1, :], o_pss[1][0:1, :], AF.Ln)
                    rcb = p_ms.tile([65, 512], bf16, name="rcb", tag="ms")
                    nc.scalar.activation(rcb[64:65, :], lnd[64:65, :], AF.Exp, scale=-1.0)
                    nc.scalar.activation(rcb[0:1, :], lnd[0:1, :], AF.Exp, scale=-1.0)
                    if pend is not None:
                        Ohs.append(emit_norm(pend))
                    pend = (o_pss, rcb, qs)
                Ohs.append(emit_norm(pend))
                for mt in range(4):
                    ps = pA.tile([128, 512], f32, name="ps_op", tag="pa")
                    for hb in range(4):
                        nc.tensor.matmul(ps[:], w_attn[:, widx + 3, hb, mt * 128:(mt + 1) * 128],
                                         Ohs[hb][:], start=(hb == 0), stop=(hb == 3))
                    nc.vector.scalar_tensor_tensor(resid_out[:, mt, qs], ps[:],
                                                   b_attn[:, widx + 3, mt:mt + 1],
                                                   resid_in[:, mt, qs], OP.add, OP.add)

        def moe(xn3, resid_in, resid_out):
            for qc in range(2):
                qs = slice(qc * 512, (qc + 1) * 512)
                eos = [pOp.tile([128, 512], f32, name=f"eo{mt}", tag="op") for mt in range(4)]
                for e in range(2):
                    for j in range(16):
                        bv_i = e * 32 + j
                        bg_i = e * 32 + 16 + j
                        wo_i = e * 16 + j
                        wv_t = p_wi.tile([128, 512], bf16, name="wv_t", tag="wi")
                        dma(out=wv_t[:], in_=wi_h[bv_i, :, :])
                        wg_t = p_wi.tile([128, 512], bf16, name="wg_t", tag="wi")
                        dma(out=wg_t[:], in_=wi_h[bg_i, :, :])
                        wo_t = p_wo.tile([128, 512], bf16, name="wo_t", tag="wo")
                        dma(out=wo_t[:], in_=wo_h[wo_i, :, :])
                        gate_ps = pA.tile([128, 512], f32, name="gate_ps", tag="pa")
                        for kt in range(4):
                            nc.tensor.matmul(gate_ps[:], wg_t[:, kt * 128:(kt + 1) * 128],
                                             xn3[:, kt, qs], start=(kt == 0), stop=(kt == 3))
                        sg = p_eT.tile([128, 512], bf16, name="sg", tag="et")
                        nc.scalar.activation(sg[:], gate_ps[:], AF.Silu, bias=bi_sb[:, bg_i:bg_i + 1])
                        val_ps = pA.tile([128, 512], f32, name="val_ps", tag="pa")
                        for kt in range(4):
                            nc.tensor.matmul(val_ps[:], wv_t[:, kt * 128:(kt + 1) * 128],
                                             xn3[:, kt, qs], start=(kt == 0), stop=(kt == 3))
                        hid = p_hid.tile([128, 512], bf16, name="hid", tag="hid")
                        nc.vector.scalar_tensor_tensor(hid[:], val_ps[:], bi_sb[:, bv_i:bv_i + 1],
                                                       sg[:], OP.add, OP.mult)
                        first = (e == 0 and j == 0)
                        last = (e == 1 and j == 15)
                        for mt in range(4):
                            nc.tensor.matmul(eos[mt][:], wo_t[:, mt * 128:(mt + 1) * 128],
                                             hid[:], start=first, stop=last)
                for mt in range(4):
                    nc.vector.scalar_tensor_tensor(resid_out[:, mt, qs], eos[mt][:],
                                                   bo_sb[:, mt:mt + 1],
                                                   resid_in[:, mt, qs], OP.add, OP.add)

        xn1 = p_xn.tile([128, 4, 1024], bf16, name="xn1", tag="xn")
        layer_norm(X_T, xn1)
        X2 = p_resid.tile([128, 4, 1024], f32, name="X2", tag="resid")
        attention(0, xn1, scene_T, 512, X_T, X2)
        xn2 = p_xn.tile([128, 4, 1024], bf16, name="xn2", tag="xn")
        layer_norm(X2, xn2)
        X3 = p_resid.tile([128, 4, 1024], f32, name="X3", tag="resid")
        attention(4, xn2, xn2, 1024, X2, X3)
        xn3 = p_xn.tile([128, 4, 1024], bf16, name="xn3", tag="xn")
        layer_norm(X3, xn3)
        OUT_T = p_resid.tile([128, 4, 1024], f32, name="OUT_T", tag="resid")
        moe(xn3, X3, OUT_T)

        out_r = out_h.rearrange("(t p) d -> p t d", p=128)
        for tq in range(8):
            orm = p_orm.tile([128, 512], f32, name="orm", tag="orm")
            for mt in range(4):
                tp = pB.tile([128, 512], f32, name="tp", tag="pb")
                nc.tensor.transpose(tp[:, 0:128],
                                    OUT_T[:, mt, tq * 128:(tq + 1) * 128], ident[:])
                if mt % 2 == 0:
                    nc.scalar.activation(orm[:, mt * 128:(mt + 1) * 128],
                                         tp[:, 0:128], AF.Copy)
                else:
                    nc.vector.tensor_scalar_mul(orm[:, mt * 128:(mt + 1) * 128],
                                                tp[:, 0:128], 1.0)
            dma(out=out_r[:, tq, :], in_=orm[:])

    _legalize_waits(nc)
    _NC = nc
    return nc


def _legalize_waits(nc):
    # DMACopy/Matmult/Ldweights hardware encodings hold a single sem wait
    # (transpose matmuls and direct2d DMAs have one EVENTS slot); move the
    # extras onto standalone EventSemaphore instructions issued just before
    # on the same engine queue.
    from concourse import mybir
    n = 0
    for fn in nc.m.functions:
        for blk in fn.blocks:
            out = []
            for inst in blk.instructions:
                si = getattr(inst, "sync_info", None)
                ow = list(si.on_wait) if si is not None else []
                if len(ow) > 1 and getattr(inst, "opcode", None) is not None:
                    for j, w in enumerate(ow[:-1]):
                        out.append(mybir.InstEventSemaphore(
                            name=f"{inst.name}-wx{j}",
                            engine=inst.engine,
                            sync_info=mybir.SyncInfo(on_wait=[w], on_update=[]),
                        ))
                        n += 1
                    inst.sync_info = mybir.SyncInfo(
                        on_wait=[ow[-1]], on_update=list(si.on_update))
                out.append(inst)
            blk.instructions = out
    return n


def _silu(v):
    return v / (1.0 + np.exp(-v))


def _softmax(v):
    m = v.max(axis=-1, keepdims=True)
    ex = np.exp(v - m)
    return ex / ex.sum(axis=-1, keepdims=True)


def _prepare(inputs):
    inp = {k: np.asarray(v, dtype=np.float32) for k, v in inputs.items()}
    x = inp["x"]
    scene = inp["scene_tokens"]
    t = inp["t"]
    g = inp["scene_norm_g"]
    bvec = inp["scene_norm_b"]

    half = D // 2
    freqs = np.exp(-math.log(10000.0) * np.arange(half, dtype=np.float32) / (half - 1)).astype(np.float32)
    ang = t[:, None] * freqs[None, :]
    temb = np.concatenate([np.cos(ang), np.sin(ang)], axis=-1).astype(np.float32)
    ncv = _silu(temb @ inp["ne_w1"] + inp["ne_b1"]) @ inp["ne_w2"] + inp["ne_b2"]

    mod1 = ncv @ inp["ncsa_mod_w"] + inp["ncsa_mod_b"]
    shift1, scale1 = mod1[:, :D], mod1[:, D:]
    mod2 = ncv @ inp["moe_mod_w"] + inp["moe_mod_b"]
    shift2, scale2 = mod2[:, :D], mod2[:, D:]

    probs = _softmax(ncv @ inp["router_w"])
    ti = np.argsort(-probs, axis=-1, kind="stable")[:, :K]
    tw = np.take_along_axis(probs, ti, axis=-1)
    tw = tw / np.clip(tw.sum(-1, keepdims=True), 1e-8, None)

    ca_wq_e = g[:, None] * inp["ca_wq"]
    ca_bq_e = inp["ca_bq"] + bvec @ inp["ca_wq"]
    ca_bo_e = inp["ca_bo"] + inp["ca_bv"] @ inp["ca_wo"]
    zero = np.zeros(D, np.float32)
    # even heads first, then odd heads (see V layout note in _build)
    vperm = np.concatenate([np.arange(h * HD, (h + 1) * HD) for h in (0, 2, 4, 6, 1, 3, 5, 7)])

    in_maps = []
    for b in range(B):
        s1 = 1.0 + scale1[b]
        sa_wq_e = s1[:, None] * inp["sa_wq"]
        sa_bq_e = inp["sa_bq"] + shift1[b] @ inp["sa_wq"]
        sa_wk_e = s1[:, None] * inp["sa_wk"]
        sa_bk_e = inp["sa_bk"] + shift1[b] @ inp["sa_wk"]
        sa_wv_e = s1[:, None] * inp["sa_wv"]
        sa_bv_e = inp["sa_bv"] + shift1[b] @ inp["sa_wv"]
        sa_bo_e = inp["sa_bo"] + sa_bv_e @ inp["sa_wo"]

        wattn = np.stack([ca_wq_e, inp["ca_wk"], inp["ca_wv"][:, vperm], inp["ca_wo"],
                          sa_wq_e, sa_wk_e, sa_wv_e[:, vperm], inp["sa_wo"]]).astype(BF16)
        battn = np.stack([ca_bq_e, inp["ca_bk"], zero, ca_bo_e,
                          sa_bq_e, sa_bk_e, zero, sa_bo_e]).astype(np.float32)

        s2 = 1.0 + scale2[b]
        Wis, bis, Wos = [], [], []
        bo_moe = np.zeros(D, np.float32)
        for k in range(K):
            eidx = int(ti[b, k])
            w = np.float32(tw[b, k])
            Wi_e = inp["fc_in_w"][eidx]
            Wis.append(s2[:, None] * Wi_e)
            bis.append(inp["fc_in_b"][eidx] + shift2[b] @ Wi_e)
            Wos.append(w * inp["fc_out_w"][eidx])
            bo_moe = bo_moe + w * inp["fc_out_b"][eidx]
        Wi_cat = np.concatenate(Wis, axis=1)
        bi_cat = np.concatenate(bis, axis=0)
        Wo_cat = np.concatenate(Wos, axis=0)

        wi_pt = np.ascontiguousarray(
            Wi_cat.reshape(4, 128, 64, 128).transpose(2, 1, 0, 3).reshape(64, 128, 512)).astype(BF16)
        wo_pt = np.ascontiguousarray(Wo_cat.reshape(32, 128, 512)).astype(BF16)
        bi_pt = np.ascontiguousarray(bi_cat.reshape(64, 128)).astype(np.float32)
        bo_pt = np.ascontiguousarray(bo_moe.reshape(4, 128)).astype(np.float32)

        in_maps.append({
            "x": np.ascontiguousarray(x[b]),
            "scene": np.ascontiguousarray(scene[b]),
            "wattn": np.ascontiguousarray(wattn),
            "battn": np.ascontiguousarray(battn),
            "wi": wi_pt,
            "bi_t": bi_pt,
            "wo": wo_pt,
            "bo_t": bo_pt,
        })
    return in_maps


def _run(in_maps, trace=False):
    from concourse.bass_utils import run_bass_kernel_spmd
    nc = _build()
    return run_bass_kernel_spmd(nc, in_maps, list(range(NCORES)), trace=trace)


def kernel(**inputs):
    in_maps = _prepare(inputs)
    res = _run(in_maps)
    return np.stack([np.asarray(res.results[i]["out"], dtype=np.float32) for i in range(B)])


# revision 12
# speedup vs baseline: 1.8081x; 1.0457x over previous
import math
import sys

for _p in ("/root/.axon_site", "/root/.axon_site/_ro/trn_rl_repo", "/opt/trn_rl_repo"):
    if _p not in sys.path:
        sys.path.append(_p)

import numpy as np
import ml_dtypes

BF16 = ml_dtypes.bfloat16
F8 = ml_dtypes.float8_e4m3  # IEEE-style e4m3: max 240 == TRN FP8_EXP4

B, L, NS = 8, 1024, 512
D, NH, DN = 512, 8, 256
E, K, HE = 8, 2, 2048
HD = D // NH
EPS = 1e-5
NCORES = 8

SW = 64.0    # attention weight fp8 scale
SI = 64.0    # moe gate fc_in scale
SV = 32.0    # moe val fc_in scale
SO = 64.0    # moe fc_out scale

_NC = None


def _build():
    global _NC
    if _NC is not None:
        return _NC
    from concourse import bass, tile, mybir, masks

    f32 = mybir.dt.float32
    bf16 = mybir.dt.bfloat16
    f8e4 = mybir.dt.float8e4
    AF = mybir.ActivationFunctionType
    OP = mybir.AluOpType
    DR = mybir.MatmulPerfMode.DoubleRow

    nc = bass.Bass()
    x_h = nc.declare_dram_parameter("x", [L, D], bf16, isOutput=False)
    scene_h = nc.declare_dram_parameter("scene", [NS, D], bf16, isOutput=False)
    wattn_h = nc.declare_dram_parameter("wattn", [8, D, D], f8e4, isOutput=False)
    battn_h = nc.declare_dram_parameter("battn", [8, D], f32, isOutput=False)
    brow_h = nc.declare_dram_parameter("brow", [1, 1024], bf16, isOutput=False)
    wi_h = nc.declare_dram_parameter("wi", [64, 128, 512], f8e4, isOutput=False)
    bi_h = nc.declare_dram_parameter("bi_t", [64, 128], f32, isOutput=False)
    wo_h = nc.declare_dram_parameter("wo", [16, 128, 1024], f8e4, isOutput=False)
    borow_h = nc.declare_dram_parameter("borow", [1, 512], bf16, isOutput=False)
    out_h = nc.declare_dram_parameter("out", [L, D], f32, isOutput=True)

    with tile.TileContext(nc) as tc, \
         tc.tile_pool(name="sing", bufs=1) as sing, \
         tc.tile_pool(name="p_rm", bufs=2) as p_rm, \
         tc.tile_pool(name="p_resid", bufs=2) as p_resid, \
         tc.tile_pool(name="p_xn", bufs=2) as p_xn, \
         tc.tile_pool(name="p_qkv", bufs=1) as p_qkv, \
         tc.tile_pool(name="p_oh", bufs=2) as p_oh, \
         tc.tile_pool(name="p_eT", bufs=4) as p_eT, \
         tc.tile_pool(name="p_ms", bufs=8) as p_ms, \
         tc.tile_pool(name="p_msx", bufs=4) as p_msx, \
         tc.tile_pool(name="p_xb", bufs=2) as p_xb, \
         tc.tile_pool(name="p_wi", bufs=6) as p_wi, \
         tc.tile_pool(name="p_wo", bufs=2) as p_wo, \
         tc.tile_pool(name="p_x3r", bufs=8) as p_x3r, \
         tc.tile_pool(name="p_orm", bufs=2) as p_orm, \
         tc.tile_pool(name="pS", bufs=2, space="PSUM") as pS, \
         tc.tile_pool(name="pOp", bufs=3, space="PSUM") as pOp, \
         tc.tile_pool(name="pB", bufs=1, space="PSUM") as pB:

        dma = nc.sync.dma_start

        ident = sing.tile([128, 128], f32, name="ident", tag="ident")
        masks.make_identity(nc, ident[:])
        ones_bf = sing.tile([128, 1], bf16, name="ones_bf", tag="ones")
        nc.vector.memset(ones_bf[:], 1.0)
        ones_sq = sing.tile([128, 64], bf16, name="ones_sq", tag="ones_sq")
        nc.vector.memset(ones_sq[:], 1.0)
        ones_q = sing.tile([1, 512], bf16, name="ones_q", tag="ones_qq")
        nc.vector.memset(ones_q[:], 1.0)
        ones_row = sing.tile([1, 128], bf16, name="ones_row", tag="ones_r")
        nc.vector.memset(ones_row[:], 1.0)
        ones_big = sing.tile([128, 512], bf16, name="ones_big", tag="ones_b")
        nc.vector.memset(ones_big[:], 1.0)

        # HAM warm-up spins + ln/exp table preload during the DMA window
        dummy = p_ms.tile([1, 1], f32, name="dummy", tag="ms")
        nc.scalar.activation(dummy[:], ident[0:1, 0:1], AF.Ln)
        dummy2 = p_ms.tile([1, 1], f32, name="dummy2", tag="ms")
        nc.scalar.activation(dummy2[:], ident[0:1, 0:1], AF.Exp)
        for _w in range(40):
            spin = pS.tile([128, 2, 512], f32, name="spin", tag="s2")
            nc.tensor.matmul(spin[0:64, 0, :], ones_sq[:, :], ones_big[:],
                             start=True, stop=True)

        # x -> feature-major bf16 spine directly via DMA crossbar transpose
        X_T = p_resid.tile([128, 4, 1024], bf16, name="X_T", tag="resid")
        for mt in range(4):
            nc.sync.dma_start_transpose(X_T[:, mt, :],
                                        x_h[:, mt * 128:(mt + 1) * 128])
        scene_Tb = sing.tile([128, 4, 512], bf16, name="scene_Tb", tag="scTb")
        for mt in range(4):
            nc.sync.dma_start_transpose(scene_Tb[:, mt, :],
                                        scene_h[:, mt * 128:(mt + 1) * 128])
        w_attn = sing.tile([128, 8, 4, 512], f8e4, name="w_attn", tag="w_attn")
        dma(out=w_attn[:], in_=wattn_h.rearrange("i (t p) d -> p i t d", p=128))
        b_attn = sing.tile([128, 8, 4], f32, name="b_attn", tag="b_attn")
        dma(out=b_attn[:], in_=battn_h.rearrange("i (t p) -> p i t", p=128))
        brow = sing.tile([1, 1024], bf16, name="brow", tag="brow")
        dma(out=brow[:], in_=brow_h[:, :])
        borow = sing.tile([1, 512], bf16, name="borow", tag="borow")
        dma(out=borow[:], in_=borow_h[:, :])
        bi_sb = sing.tile([128, 64], f32, name="bi_sb", tag="bi_sb")
        dma(out=bi_sb[:], in_=bi_h.rearrange("b p -> p b"))

        scene_T = sing.tile([128, 4, 512], f8e4, name="scene_T", tag="scene_T")
        nc.vector.tensor_scalar_mul(scene_T[:], scene_Tb[:], 1.0)

        def layer_norm(src, xn):
            for qc in range(2):
                qs = slice(qc * 512, (qc + 1) * 512)
                sq = p_xb.tile([128, 4, 512], bf16, name="sq", tag="xbsq")
                nc.scalar.activation(sq[:], src[:, :, qs], AF.Square)
                sum_ps = pOp.tile([1, 512], f32, name="sum_ps", tag="op")
                for kt in range(4):
                    nc.tensor.matmul(sum_ps[:], ones_bf[:], src[:, kt, qs],
                                     start=(kt == 0), stop=(kt == 3))
                sq_ps = pOp.tile([1, 512], f32, name="sq_ps", tag="op")
                for kt in range(4):
                    nc.tensor.matmul(sq_ps[:], ones_bf[:], sq[:, kt, :],
                                     start=(kt == 0), stop=(kt == 3))
                m_bf = p_ms.tile([1, 512], bf16, name="m_bf", tag="ms")
                nc.vector.tensor_scalar_mul(m_bf[:], sum_ps[:], 1.0 / 512.0)
                m = p_ms.tile([1, 512], f32, name="m", tag="ms")
                nc.vector.tensor_scalar_mul(m[:], sum_ps[:], 1.0 / 512.0)
                e2 = p_ms.tile([1, 512], f32, name="e2", tag="ms")
                nc.vector.tensor_scalar(e2[:], sq_ps[:], 1.0 / 512.0, EPS, OP.mult, OP.add)
                mm = p_ms.tile([1, 512], f32, name="mm", tag="ms")
                nc.vector.tensor_tensor(mm[:], m[:], m[:], OP.mult)
                var = p_ms.tile([1, 512], f32, name="var", tag="ms")
                nc.vector.tensor_tensor(var[:], e2[:], mm[:], OP.subtract)
                # 1/sqrt(var) = exp(-0.5 ln var): stays in the ln/exp table set
                lnv = p_ms.tile([1, 512], f32, name="lnv", tag="ms")
                nc.scalar.activation(lnv[:], var[:], AF.Ln)
                r_bf = p_ms.tile([1, 512], bf16, name="r_bf", tag="ms")
                nc.scalar.activation(r_bf[:], lnv[:], AF.Exp, scale=-0.5)
                rb_ps = pB.tile([128, 512], f32, name="rb_ps", tag="pb")
                nc.tensor.matmul(rb_ps[:], ones_row[:], r_bf[:], start=True, stop=True)
                mb_ps = pOp.tile([128, 512], f32, name="mb_ps", tag="op")
                nc.tensor.matmul(mb_ps[:], ones_row[:], m_bf[:], start=True, stop=True)
                for mt in range(4):
                    xs = p_msx.tile([128, 512], bf16, name="xs", tag="msx")
                    nc.vector.tensor_tensor(xs[:], src[:, mt, qs], mb_ps[:], OP.subtract)
                    nc.vector.tensor_tensor(xn[:, mt, qs], xs[:], rb_ps[:], OP.mult)

        def attention(widx, xq_T, kv_T, kv_len, resid_in, resid_out):
            nkp = kv_len // 128
            nkc = kv_len // 512
            # fp8 DoubleRow projections: psum = 64*W @ x
            Q_T = p_qkv.tile([128, 4, 1024], bf16, name="Q_T", tag="q")
            for mt in range(4):
                q_pss = [pOp.tile([128, 512], f32, name="q_ps", tag="op")
                         for _ in range(2)]
                for t2 in range(2):
                    for qc in range(2):
                        nc.tensor.matmul(q_pss[qc][:],
                                         w_attn[:, widx, 2 * t2:2 * t2 + 2, mt * 128:(mt + 1) * 128],
                                         xq_T[:, 2 * t2:2 * t2 + 2, qc * 512:(qc + 1) * 512],
                                         start=(t2 == 0), stop=(t2 == 1), perf_mode=DR)
                for qc in range(2):
                    nc.vector.tensor_scalar_add(Q_T[:, mt, qc * 512:(qc + 1) * 512],
                                                q_pss[qc][:],
                                                b_attn[:, widx, mt:mt + 1])
            K_T = p_qkv.tile([128, 4, 1024], bf16, name="K_T", tag="k")
            for mt in range(4):
                k_pss = [pOp.tile([128, 512], f32, name="k_ps", tag="op")
                         for _ in range(nkc)]
                for t2 in range(2):
                    for kc in range(nkc):
                        nc.tensor.matmul(k_pss[kc][:],
                                         w_attn[:, widx + 1, 2 * t2:2 * t2 + 2, mt * 128:(mt + 1) * 128],
                                         kv_T[:, 2 * t2:2 * t2 + 2, kc * 512:(kc + 1) * 512],
                                         start=(t2 == 0), stop=(t2 == 1), perf_mode=DR)
                for kc in range(nkc):
                    nc.vector.tensor_scalar_add(K_T[:, mt, kc * 512:(kc + 1) * 512],
                                                k_pss[kc][:],
                                                b_attn[:, widx + 1, mt:mt + 1])
            # V in fp8 (64x), denominator ones packed; pad 65->68 for DR stride
            V_ev = p_qkv.tile([128, 8, 4, 68], f8e4, name="V_ev", tag="ve")
            V_od = p_qkv.tile([128, 8, 4, 128], f8e4, name="V_od", tag="vo")
            nc.vector.memset(V_ev[:, 0:nkp, :, 64:65], 1.0)
            nc.vector.memset(V_od[:, 0:nkp, :, 0:1], 1.0)
            nc.vector.memset(V_od[:, 0:nkp, :, 1:64], 0.0)
            for kp in range(nkp):
                v_ps = pOp.tile([128, 2, 4, 64], f32, name="v_ps", tag="op")
                for t2 in range(2):
                    nc.tensor.matmul(v_ps[:],
                                     kv_T[:, 2 * t2:2 * t2 + 2, kp * 128:(kp + 1) * 128],
                                     w_attn[:, widx + 2, 2 * t2:2 * t2 + 2, :],
                                     start=(t2 == 0), stop=(t2 == 1), perf_mode=DR)
                nc.vector.tensor_scalar_mul(V_ev[:, kp, :, 0:64], v_ps[:, 0, :, :], 1.0)
                nc.vector.tensor_scalar_mul(V_od[:, kp, :, 64:128], v_ps[:, 1, :, :], 1.0)

            def emit_norm(st):
                o_pss, rcb, hb, Oh_all = st
                rb_ps = pB.tile([128, 512], f32, name="rb_ps", tag="pb")
                nc.tensor.matmul(rb_ps[0:64, :], ones_sq[64:65, 0:64], rcb[64:65, :],
                                 start=True, stop=True)
                nc.tensor.matmul(rb_ps[64:128, :], ones_sq[0:1, 0:64], rcb[0:1, :],
                                 start=True, stop=True)
                rb_sb = p_eT.tile([128, 512], bf16, name="rb_sb", tag="rbs")
                nc.vector.tensor_scalar_mul(rb_sb[:], rb_ps[:], 1.0)
                nc.vector.tensor_tensor(Oh_all[0:64, hb, :], o_pss[0][0:64, :],
                                        rb_sb[0:64, :], OP.mult)
                nc.vector.tensor_tensor(Oh_all[64:128, hb, :], o_pss[1][64:128, :],
                                        rb_sb[64:128, :], OP.mult)

            for qc in range(2):
                qs = slice(qc * 512, (qc + 1) * 512)
                # Oh_all = 64 * attn_out per head, fp8
                Oh_all = p_oh.tile([128, 4, 512], f8e4, name="Oh_all", tag="oh")
                pend = None
                for hb in range(4):
                    o_pss = []
                    for par in range(2):
                        po = par * 64
                        o_ps = pOp.tile([128, 512], f32, name="o_ps", tag="op")
                        for b2 in range(nkp // 2):
                            s2 = pS.tile([128, 2, 512], f32, name="s2", tag="s2")
                            nc.tensor.matmul(s2[0:64, 0, :], ones_sq[:, :], ones_big[:],
                                             start=True, stop=True)
                            for k2 in range(2):
                                kp = 2 * b2 + k2
                                nc.tensor.matmul(s2[:, k2, :],
                                                 K_T[po:po + 64, hb, kp * 128:(kp + 1) * 128],
                                                 Q_T[po:po + 64, hb, qs],
                                                 start=True, stop=True)
                            e2 = p_eT.tile([128, 2, 512], f8e4, name="e2", tag="et")
                            nc.scalar.activation(e2[:], s2[:], AF.Exp, scale=0.125 / 4096.0)
                            if par == 0:
                                nc.tensor.matmul(o_ps[0:65, :], V_ev[:, 2 * b2:2 * b2 + 2, hb, 0:65],
                                                 e2[:], start=(b2 == 0), stop=(b2 == nkp // 2 - 1),
                                                 perf_mode=DR)
                            else:
                                nc.tensor.matmul(o_ps[:], V_od[:, 2 * b2:2 * b2 + 2, hb, :],
                                                 e2[:], start=(b2 == 0), stop=(b2 == nkp // 2 - 1),
                                                 perf_mode=DR)
                        o_pss.append(o_ps)
                    # softmax denominators: 1/d = exp(-ln d), ln/exp table set
                    lnd = p_ms.tile([65, 512], f32, name="lnd", tag="ms")
                    nc.scalar.activation(lnd[64:65, :], o_pss[0][64:65, :], AF.Ln)
                    nc.scalar.activation(lnd[0:1, :], o_pss[1][0:1, :], AF.Ln)
                    rcb = p_ms.tile([65, 512], bf16, name="rcb", tag="ms")
                    nc.scalar.activation(rcb[64:65, :], lnd[64:65, :], AF.Exp, scale=-1.0)
                    nc.scalar.activation(rcb[0:1, :], lnd[0:1, :], AF.Exp, scale=-1.0)
                    if pend is not None:
                        emit_norm(pend)
                    pend = (o_pss, rcb, hb, Oh_all)
                emit_norm(pend)
                # o-proj: psum = (64 w)(64 attn) + 4096*bias -> /4096 + resid
                for mt in range(4):
                    ps = pOp.tile([128, 512], f32, name="ps_op", tag="op")
                    nc.tensor.matmul(ps[:], brow[0:1, (widx // 4) * 512 + mt * 128:
                                                (widx // 4) * 512 + (mt + 1) * 128],
                                     ones_q[:], start=True, stop=False)
                    for h2 in range(2):
                        nc.tensor.matmul(ps[:],
                                         w_attn[:, widx + 3, 2 * h2:2 * h2 + 2, mt * 128:(mt + 1) * 128],
                                         Oh_all[:, 2 * h2:2 * h2 + 2, :],
                                         start=False, stop=(h2 == 1), perf_mode=DR)
                    nc.vector.scalar_tensor_tensor(resid_out[:, mt, qs], ps[:],
                                                   1.0 / 4096.0,
                                                   resid_in[:, mt, qs], OP.mult, OP.add)

        def moe(xn3, X3):
            # pass 1: hid = SV * (val + bv) * silu(gate + bg), fp8 into arena
            hid_ar = sing.tile([128, 32, 1024], f8e4, name="hid_ar", tag="hid")
            for j in range(32):
                e, jj = j // 16, j % 16
                bv_i = e * 32 + jj
                bg_i = e * 32 + 16 + jj
                wv_t = p_wi.tile([128, 4, 128], f8e4, name="wv_t", tag="wi")
                dma(out=wv_t[:], in_=wi_h[bv_i, :, :].rearrange("p (t m) -> p t m", t=4))
                wg_t = p_wi.tile([128, 4, 128], f8e4, name="wg_t", tag="wi")
                dma(out=wg_t[:], in_=wi_h[bg_i, :, :].rearrange("p (t m) -> p t m", t=4))
                qsl = [slice(0, 512), slice(512, 1024)]
                gate_pss = [pOp.tile([128, 512], f32, name="gate_ps", tag="op")
                            for _ in range(2)]
                for t2 in range(2):
                    for qc in range(2):
                        nc.tensor.matmul(gate_pss[qc][:], wg_t[:, 2 * t2:2 * t2 + 2, :],
                                         xn3[:, 2 * t2:2 * t2 + 2, qsl[qc]],
                                         start=(t2 == 0), stop=(t2 == 1), perf_mode=DR)
                sgs = []
                for qc in range(2):
                    sg = p_eT.tile([128, 512], bf16, name="sg", tag="et")
                    nc.scalar.activation(sg[:], gate_pss[qc][:], AF.Silu,
                                         bias=bi_sb[:, bg_i:bg_i + 1], scale=1.0 / SI)
                    sgs.append(sg)
                val_pss = [pOp.tile([128, 512], f32, name="val_ps", tag="op")
                           for _ in range(2)]
                for t2 in range(2):
                    for qc in range(2):
                        nc.tensor.matmul(val_pss[qc][:], wv_t[:, 2 * t2:2 * t2 + 2, :],
                                         xn3[:, 2 * t2:2 * t2 + 2, qsl[qc]],
                                         start=(t2 == 0), stop=(t2 == 1), perf_mode=DR)
                for qc in range(2):
                    nc.vector.scalar_tensor_tensor(hid_ar[:, j, qsl[qc]], val_pss[qc][:],
                                                   bi_sb[:, bv_i:bv_i + 1],
                                                   sgs[qc][:], OP.add, OP.mult)
                # X3 row-major staging via DMA crossbar transpose (no PE)
                if j < 16 and j % 2 == 0:
                    tq = j // 2
                    x3r = p_x3r.tile([128, 512], bf16, name=f"x3r{tq}", tag="x3r")
                    for mt in range(4):
                        nc.sync.dma_start_transpose(
                            x3r[:, mt * 128:(mt + 1) * 128],
                            X3[:, mt, tq * 128:(tq + 1) * 128])
                    yield_tiles.append(x3r)

            # pass 2: token-major out-proj: eo[tok, d] = sum_he hid[he, tok]*wo[he, d]
            out_r = out_h.rearrange("(t p) d -> p t d", p=128)
            for qc in range(2):
                eos_a = pS.tile([128, 2, 512], f32, name="eos_a", tag="s2")
                eos_b = pS.tile([128, 2, 512], f32, name="eos_b", tag="s2")
                eslice = [eos_a[:, 0, :], eos_a[:, 1, :], eos_b[:, 0, :], eos_b[:, 1, :]]
                for tc in range(4):
                    nc.tensor.matmul(eslice[tc], borow[:, 0:128], ones_q[:, 0:512],
                                     start=True, stop=False)
                for jp in range(16):
                    wo_t = p_wo.tile([128, 2, 512], f8e4, name="wo_t", tag="wo")
                    dma(out=wo_t[:], in_=wo_h[jp, :, :].rearrange("p (t m) -> p t m", t=2))
                    for tc in range(4):
                        tok = slice(qc * 512 + tc * 128, qc * 512 + (tc + 1) * 128)
                        nc.tensor.matmul(eslice[tc],
                                         hid_ar[:, 2 * jp:2 * jp + 2, tok],
                                         wo_t[:], start=False, stop=(jp == 15),
                                         perf_mode=DR)
                for tc in range(4):
                    tq = qc * 4 + tc
                    orm = p_orm.tile([128, 512], f32, name="orm", tag="orm")
                    nc.vector.scalar_tensor_tensor(orm[:], eslice[tc], 1.0 / (SV * SO),
                                                   yield_tiles[tq][:], OP.mult, OP.add)
                    dma(out=out_r[:, tq, :], in_=orm[:])

        yield_tiles = []
        xn1 = p_xn.tile([128, 4, 1024], f8e4, name="xn1", tag="xn")
        layer_norm(X_T, xn1)
        X2 = p_resid.tile([128, 4, 1024], bf16, name="X2", tag="resid")
        attention(0, xn1, scene_T, 512, X_T, X2)
        xn2 = p_xn.tile([128, 4, 1024], f8e4, name="xn2", tag="xn")
        layer_norm(X2, xn2)
        X3 = p_resid.tile([128, 4, 1024], bf16, name="X3", tag="resid")
        attention(4, xn2, xn2, 1024, X2, X3)
        xn3 = p_xn.tile([128, 4, 1024], f8e4, name="xn3", tag="xn")
        layer_norm(X3, xn3)
        moe(xn3, X3)

    _legalize_waits(nc)
    _NC = nc
    return nc


def _legalize_waits(nc):
    # Matmult/Ldweights/DMA encodings hold a single sem wait; split extras
    # onto EventSemaphore instructions on the same queue.
    from concourse import mybir
    n = 0
    for fn in nc.m.functions:
        for blk in fn.blocks:
            out = []
            for inst in blk.instructions:
                si = getattr(inst, "sync_info", None)
                ow = list(si.on_wait) if si is not None else []
                if len(ow) > 1 and getattr(inst, "opcode", None) is not None:
                    for j, w in enumerate(ow[:-1]):
                        out.append(mybir.InstEventSemaphore(
                            name=f"{inst.name}-wx{j}",
                            engine=inst.engine,
                            sync_info=mybir.SyncInfo(on_wait=[w], on_update=[]),
                        ))
                        n += 1
                    inst.sync_info = mybir.SyncInfo(
                        on_wait=[ow[-1]], on_update=list(si.on_update))
                out.append(inst)
            blk.instructions = out
    return n


def _silu(v):
    return v / (1.0 + np.exp(-v))


def _softmax(v):
    m = v.max(axis=-1, keepdims=True)
    ex = np.exp(v - m)
    return ex / ex.sum(axis=-1, keepdims=True)


def _f8(x):
    return np.clip(x, -240.0, 240.0).astype(F8)


def _prepare(inputs):
    inp = {k: np.asarray(v, dtype=np.float32) for k, v in inputs.items()}
    x = inp["x"]
    scene = inp["scene_tokens"]
    t = inp["t"]
    g = inp["scene_norm_g"]
    bvec = inp["scene_norm_b"]

    half = D // 2
    freqs = np.exp(-math.log(10000.0) * np.arange(half, dtype=np.float32) / (half - 1)).astype(np.float32)
    ang = t[:, None] * freqs[None, :]
    temb = np.concatenate([np.cos(ang), np.sin(ang)], axis=-1).astype(np.float32)
    ncv = _silu(temb @ inp["ne_w1"] + inp["ne_b1"]) @ inp["ne_w2"] + inp["ne_b2"]

    mod1 = ncv @ inp["ncsa_mod_w"] + inp["ncsa_mod_b"]
    shift1, scale1 = mod1[:, :D], mod1[:, D:]
    mod2 = ncv @ inp["moe_mod_w"] + inp["moe_mod_b"]
    shift2, scale2 = mod2[:, :D], mod2[:, D:]

    probs = _softmax(ncv @ inp["router_w"])
    ti = np.argsort(-probs, axis=-1, kind="stable")[:, :K]
    tw = np.take_along_axis(probs, ti, axis=-1)
    tw = tw / np.clip(tw.sum(-1, keepdims=True), 1e-8, None)

    ca_wq_e = g[:, None] * inp["ca_wq"]
    ca_bq_e = inp["ca_bq"] + bvec @ inp["ca_wq"]
    ca_bo_e = inp["ca_bo"] + inp["ca_bv"] @ inp["ca_wo"]
    zero = np.zeros(D, np.float32)
    vperm = np.concatenate([np.arange(h * HD, (h + 1) * HD) for h in (0, 2, 4, 6, 1, 3, 5, 7)])

    in_maps = []
    for b in range(B):
        s1 = 1.0 + scale1[b]
        sa_wq_e = s1[:, None] * inp["sa_wq"]
        sa_bq_e = inp["sa_bq"] + shift1[b] @ inp["sa_wq"]
        sa_wk_e = s1[:, None] * inp["sa_wk"]
        sa_bk_e = inp["sa_bk"] + shift1[b] @ inp["sa_wk"]
        sa_wv_e = s1[:, None] * inp["sa_wv"]
        sa_bv_e = inp["sa_bv"] + shift1[b] @ inp["sa_wv"]
        sa_bo_e = inp["sa_bo"] + sa_bv_e @ inp["sa_wo"]

        wattn = _f8(SW * np.stack([ca_wq_e, inp["ca_wk"], inp["ca_wv"][:, vperm], inp["ca_wo"],
                                   sa_wq_e, sa_wk_e, sa_wv_e[:, vperm], inp["sa_wo"]]))
        # Q/K biases pre-scaled by SW (psum is 64x); V zero; O via brow
        battn = SW * np.stack([ca_bq_e, inp["ca_bk"], zero, zero,
                               sa_bq_e, sa_bk_e, zero, zero]).astype(np.float32)
        brow_v = np.zeros((1, 1024), np.float32)
        brow_v[0, 0:512] = SW * SW * ca_bo_e
        brow_v[0, 512:1024] = SW * SW * sa_bo_e

        s2 = 1.0 + scale2[b]
        Wis, bis, Wos = [], [], []
        bo_moe = np.zeros(D, np.float32)
        for k in range(K):
            eidx = int(ti[b, k])
            w = np.float32(tw[b, k])
            Wi_e = inp["fc_in_w"][eidx]
            Wis.append(s2[:, None] * Wi_e)
            bis.append(inp["fc_in_b"][eidx] + shift2[b] @ Wi_e)
            Wos.append(w * inp["fc_out_w"][eidx])
            bo_moe = bo_moe + w * inp["fc_out_b"][eidx]
        Wi_cat = np.concatenate(Wis, axis=1)          # [D, 2*2HE] = [512, 8192]
        bi_cat = np.concatenate(bis, axis=0)          # [8192]
        Wo_cat = np.concatenate(Wos, axis=0)          # [2*HE, D] = [4096, 512]

        # wi chunks [64][d_lo 128][d_hi 4 * h_lo 128]; val chunks x SV, gate x SI
        wi_pt = np.ascontiguousarray(
            Wi_cat.reshape(4, 128, 64, 128).transpose(2, 1, 0, 3).reshape(64, 128, 512))
        wsc = np.zeros((64, 1, 1), np.float32)
        bsc = np.zeros(64, np.float32)
        for e in range(2):
            wsc[e * 32:e * 32 + 16] = SV
            wsc[e * 32 + 16:e * 32 + 32] = SI
            bsc[e * 32:e * 32 + 16] = SV
            bsc[e * 32 + 16:e * 32 + 32] = 1.0
        wi8 = _f8(wi_pt * wsc)
        bi_pt = np.ascontiguousarray(bi_cat.reshape(64, 128) * bsc[:, None]).astype(np.float32)
        # wo chunks [16][he_lo 128][he_hi 2 * d 512], x SO
        wo8 = _f8(SO * np.ascontiguousarray(
            Wo_cat.reshape(16, 2, 128, 512).transpose(0, 2, 1, 3).reshape(16, 128, 1024)))
        borow_v = (SV * SO * bo_moe).reshape(1, 512)

        in_maps.append({
            "x": np.ascontiguousarray(x[b]).astype(BF16),
            "scene": np.ascontiguousarray(scene[b]).astype(BF16),
            "wattn": np.ascontiguousarray(wattn),
            "battn": np.ascontiguousarray(battn),
            "brow": brow_v.astype(BF16),
            "wi": wi8,
            "bi_t": bi_pt,
            "wo": wo8,
            "borow": borow_v.astype(BF16),
        })
    return in_maps


def _run(in_maps, trace=False):
    from concourse.bass_utils import run_bass_kernel_spmd
    nc = _build()
    return run_bass_kernel_spmd(nc, in_maps, list(range(NCORES)), trace=trace)


def kernel(**inputs):
    in_maps = _prepare(inputs)
    res = _run(in_maps)
    return np.stack([np.asarray(res.results[i]["out"], dtype=np.float32) for i in range(B)])


# revision 13
# speedup vs baseline: 1.8532x; 1.0250x over previous
import math
import sys

for _p in ("/root/.axon_site", "/root/.axon_site/_ro/trn_rl_repo", "/opt/trn_rl_repo"):
    if _p not in sys.path:
        sys.path.append(_p)

import numpy as np
import ml_dtypes

BF16 = ml_dtypes.bfloat16
F8 = ml_dtypes.float8_e4m3  # IEEE-style e4m3: max 240 == TRN FP8_EXP4

B, L, NS = 8, 1024, 512
D, NH, DN = 512, 8, 256
E, K, HE = 8, 2, 2048
HD = D // NH
EPS = 1e-5
NCORES = 8

SW = 64.0    # attention weight fp8 scale
SI = 64.0    # moe gate fc_in scale
SV = 32.0    # moe val fc_in scale
SO = 64.0    # moe fc_out scale

_NC = None


def _build():
    global _NC
    if _NC is not None:
        return _NC
    from concourse import bass, tile, mybir, masks

    f32 = mybir.dt.float32
    bf16 = mybir.dt.bfloat16
    f8e4 = mybir.dt.float8e4
    AF = mybir.ActivationFunctionType
    OP = mybir.AluOpType
    DR = mybir.MatmulPerfMode.DoubleRow

    nc = bass.Bass()
    x_h = nc.declare_dram_parameter("x", [L, D], bf16, isOutput=False)
    scene_h = nc.declare_dram_parameter("scene", [NS, D], bf16, isOutput=False)
    wattn_h = nc.declare_dram_parameter("wattn", [8, D, D], f8e4, isOutput=False)
    battn_h = nc.declare_dram_parameter("battn", [8, D], f32, isOutput=False)
    brow_h = nc.declare_dram_parameter("brow", [1, 1024], bf16, isOutput=False)
    wi_h = nc.declare_dram_parameter("wi", [64, 128, 512], f8e4, isOutput=False)
    bi_h = nc.declare_dram_parameter("bi_t", [64, 128], f32, isOutput=False)
    wo_h = nc.declare_dram_parameter("wo", [16, 128, 1024], f8e4, isOutput=False)
    borow_h = nc.declare_dram_parameter("borow", [1, 512], bf16, isOutput=False)
    out_h = nc.declare_dram_parameter("out", [L, D], f32, isOutput=True)

    with tile.TileContext(nc) as tc, \
         tc.tile_pool(name="sing", bufs=1) as sing, \
         tc.tile_pool(name="p_rm", bufs=2) as p_rm, \
         tc.tile_pool(name="p_resid", bufs=2) as p_resid, \
         tc.tile_pool(name="p_xn", bufs=2) as p_xn, \
         tc.tile_pool(name="p_qkv", bufs=1) as p_qkv, \
         tc.tile_pool(name="p_oh", bufs=2) as p_oh, \
         tc.tile_pool(name="p_eT", bufs=4) as p_eT, \
         tc.tile_pool(name="p_ms", bufs=8) as p_ms, \
         tc.tile_pool(name="p_msx", bufs=4) as p_msx, \
         tc.tile_pool(name="p_xb", bufs=2) as p_xb, \
         tc.tile_pool(name="p_wi", bufs=6) as p_wi, \
         tc.tile_pool(name="p_wo", bufs=2) as p_wo, \
         tc.tile_pool(name="p_x3r", bufs=8) as p_x3r, \
         tc.tile_pool(name="p_orm", bufs=2) as p_orm, \
         tc.tile_pool(name="pS", bufs=2, space="PSUM") as pS, \
         tc.tile_pool(name="pOp", bufs=3, space="PSUM") as pOp, \
         tc.tile_pool(name="pB", bufs=1, space="PSUM") as pB:

        dma = nc.sync.dma_start

        ident = sing.tile([128, 128], f32, name="ident", tag="ident")
        masks.make_identity(nc, ident[:])
        ones_bf = sing.tile([128, 1], bf16, name="ones_bf", tag="ones")
        nc.vector.memset(ones_bf[:], 1.0)
        ones_sq = sing.tile([128, 64], bf16, name="ones_sq", tag="ones_sq")
        nc.vector.memset(ones_sq[:], 1.0)
        ones_q = sing.tile([1, 512], bf16, name="ones_q", tag="ones_qq")
        nc.vector.memset(ones_q[:], 1.0)
        ones_row = sing.tile([1, 128], bf16, name="ones_row", tag="ones_r")
        nc.vector.memset(ones_row[:], 1.0)
        ones_big = sing.tile([128, 512], bf16, name="ones_big", tag="ones_b")
        nc.vector.memset(ones_big[:], 1.0)

        # HAM warm-up spins + ln/exp table preload during the DMA window
        dummy = p_ms.tile([1, 1], f32, name="dummy", tag="ms")
        nc.scalar.activation(dummy[:], ident[0:1, 0:1], AF.Ln)
        dummy2 = p_ms.tile([1, 1], f32, name="dummy2", tag="ms")
        nc.scalar.activation(dummy2[:], ident[0:1, 0:1], AF.Exp)
        for _w in range(40):
            spin = pS.tile([128, 2, 512], f32, name="spin", tag="s2")
            nc.tensor.matmul(spin[0:64, 0, :], ones_sq[:, :], ones_big[:],
                             start=True, stop=True)

        # x -> feature-major bf16 spine directly via DMA crossbar transpose
        X_T = p_resid.tile([128, 4, 1024], bf16, name="X_T", tag="resid")
        for mt in range(4):
            nc.sync.dma_start_transpose(X_T[:, mt, :],
                                        x_h[:, mt * 128:(mt + 1) * 128])
        scene_Tb = sing.tile([128, 4, 512], bf16, name="scene_Tb", tag="scTb")
        for mt in range(4):
            nc.sync.dma_start_transpose(scene_Tb[:, mt, :],
                                        scene_h[:, mt * 128:(mt + 1) * 128])
        w_attn = sing.tile([128, 8, 4, 512], f8e4, name="w_attn", tag="w_attn")
        dma(out=w_attn[:], in_=wattn_h.rearrange("i (t p) d -> p i t d", p=128))
        b_attn = sing.tile([128, 8, 4], f32, name="b_attn", tag="b_attn")
        dma(out=b_attn[:], in_=battn_h.rearrange("i (t p) -> p i t", p=128))
        brow = sing.tile([1, 1024], bf16, name="brow", tag="brow")
        dma(out=brow[:], in_=brow_h[:, :])
        borow = sing.tile([1, 512], bf16, name="borow", tag="borow")
        dma(out=borow[:], in_=borow_h[:, :])
        bi_sb = sing.tile([128, 64], f32, name="bi_sb", tag="bi_sb")
        dma(out=bi_sb[:], in_=bi_h.rearrange("b p -> p b"))

        scene_T = sing.tile([128, 4, 512], f8e4, name="scene_T", tag="scene_T")
        nc.vector.tensor_scalar_mul(scene_T[:], scene_Tb[:], 1.0)

        def layer_norm(src, xn):
            for qc in range(2):
                qs = slice(qc * 512, (qc + 1) * 512)
                sq = p_xb.tile([128, 4, 512], bf16, name="sq", tag="xbsq")
                nc.scalar.activation(sq[:], src[:, :, qs], AF.Square)
                sum_ps = pOp.tile([1, 512], f32, name="sum_ps", tag="op")
                for kt in range(4):
                    nc.tensor.matmul(sum_ps[:], ones_bf[:], src[:, kt, qs],
                                     start=(kt == 0), stop=(kt == 3))
                sq_ps = pOp.tile([1, 512], f32, name="sq_ps", tag="op")
                for kt in range(4):
                    nc.tensor.matmul(sq_ps[:], ones_bf[:], sq[:, kt, :],
                                     start=(kt == 0), stop=(kt == 3))
                m_bf = p_ms.tile([1, 512], bf16, name="m_bf", tag="ms")
                nc.vector.tensor_scalar_mul(m_bf[:], sum_ps[:], 1.0 / 512.0)
                m = p_ms.tile([1, 512], f32, name="m", tag="ms")
                nc.vector.tensor_scalar_mul(m[:], sum_ps[:], 1.0 / 512.0)
                e2 = p_ms.tile([1, 512], f32, name="e2", tag="ms")
                nc.vector.tensor_scalar(e2[:], sq_ps[:], 1.0 / 512.0, EPS, OP.mult, OP.add)
                mm = p_ms.tile([1, 512], f32, name="mm", tag="ms")
                nc.vector.tensor_tensor(mm[:], m[:], m[:], OP.mult)
                var = p_ms.tile([1, 512], f32, name="var", tag="ms")
                nc.vector.tensor_tensor(var[:], e2[:], mm[:], OP.subtract)
                # 1/sqrt(var) = exp(-0.5 ln var): stays in the ln/exp table set
                lnv = p_ms.tile([1, 512], f32, name="lnv", tag="ms")
                nc.scalar.activation(lnv[:], var[:], AF.Ln)
                r_bf = p_ms.tile([1, 512], bf16, name="r_bf", tag="ms")
                nc.scalar.activation(r_bf[:], lnv[:], AF.Exp, scale=-0.5)
                rb_ps = pB.tile([128, 512], f32, name="rb_ps", tag="pb")
                nc.tensor.matmul(rb_ps[:], ones_row[:], r_bf[:], start=True, stop=True)
                mb_ps = pOp.tile([128, 512], f32, name="mb_ps", tag="op")
                nc.tensor.matmul(mb_ps[:], ones_row[:], m_bf[:], start=True, stop=True)
                for mt in range(4):
                    xs = p_msx.tile([128, 512], bf16, name="xs", tag="msx")
                    nc.vector.tensor_tensor(xs[:], src[:, mt, qs], mb_ps[:], OP.subtract)
                    nc.vector.tensor_tensor(xn[:, mt, qs], xs[:], rb_ps[:], OP.mult)

        def attention(widx, xq_T, kv_T, kv_len, resid_in, resid_out):
            nkp = kv_len // 128
            nkc = kv_len // 512
            # fp8 DoubleRow projections: psum = 64*W @ x
            Q_T = p_qkv.tile([128, 4, 1024], bf16, name="Q_T", tag="q")
            for mt in range(4):
                q_pss = [pOp.tile([128, 512], f32, name="q_ps", tag="op")
                         for _ in range(2)]
                for t2 in range(2):
                    for qc in range(2):
                        nc.tensor.matmul(q_pss[qc][:],
                                         w_attn[:, widx, 2 * t2:2 * t2 + 2, mt * 128:(mt + 1) * 128],
                                         xq_T[:, 2 * t2:2 * t2 + 2, qc * 512:(qc + 1) * 512],
                                         start=(t2 == 0), stop=(t2 == 1), perf_mode=DR)
                for qc in range(2):
                    nc.vector.tensor_scalar_add(Q_T[:, mt, qc * 512:(qc + 1) * 512],
                                                q_pss[qc][:],
                                                b_attn[:, widx, mt:mt + 1])
            K_T = p_qkv.tile([128, 4, 1024], bf16, name="K_T", tag="k")
            for mt in range(4):
                k_pss = [pOp.tile([128, 512], f32, name="k_ps", tag="op")
                         for _ in range(nkc)]
                for t2 in range(2):
                    for kc in range(nkc):
                        nc.tensor.matmul(k_pss[kc][:],
                                         w_attn[:, widx + 1, 2 * t2:2 * t2 + 2, mt * 128:(mt + 1) * 128],
                                         kv_T[:, 2 * t2:2 * t2 + 2, kc * 512:(kc + 1) * 512],
                                         start=(t2 == 0), stop=(t2 == 1), perf_mode=DR)
                for kc in range(nkc):
                    nc.vector.tensor_scalar_add(K_T[:, mt, kc * 512:(kc + 1) * 512],
                                                k_pss[kc][:],
                                                b_attn[:, widx + 1, mt:mt + 1])
            # V in fp8 (64x), denominator ones packed; pad 65->68 for DR stride
            V_ev = p_qkv.tile([128, 8, 4, 68], f8e4, name="V_ev", tag="ve")
            V_od = p_qkv.tile([128, 8, 4, 128], f8e4, name="V_od", tag="vo")
            nc.vector.memset(V_ev[:, 0:nkp, :, 64:65], 1.0)
            nc.vector.memset(V_od[:, 0:nkp, :, 0:1], 1.0)
            nc.vector.memset(V_od[:, 0:nkp, :, 1:64], 0.0)
            for kp in range(nkp):
                v_ps = pOp.tile([128, 2, 4, 64], f32, name="v_ps", tag="op")
                for t2 in range(2):
                    nc.tensor.matmul(v_ps[:],
                                     kv_T[:, 2 * t2:2 * t2 + 2, kp * 128:(kp + 1) * 128],
                                     w_attn[:, widx + 2, 2 * t2:2 * t2 + 2, :],
                                     start=(t2 == 0), stop=(t2 == 1), perf_mode=DR)
                nc.vector.tensor_scalar_mul(V_ev[:, kp, :, 0:64], v_ps[:, 0, :, :], 1.0)
                nc.vector.tensor_scalar_mul(V_od[:, kp, :, 64:128], v_ps[:, 1, :, :], 1.0)

            def emit_norm(st):
                o_pss, rcb, hb, Oh_all = st
                rb_ps = pB.tile([128, 512], f32, name="rb_ps", tag="pb")
                nc.tensor.matmul(rb_ps[0:64, :], ones_sq[64:65, 0:64], rcb[64:65, :],
                                 start=True, stop=True)
                nc.tensor.matmul(rb_ps[64:128, :], ones_sq[0:1, 0:64], rcb[0:1, :],
                                 start=True, stop=True)
                rb_sb = p_eT.tile([128, 512], bf16, name="rb_sb", tag="rbs")
                nc.vector.tensor_scalar_mul(rb_sb[:], rb_ps[:], 1.0)
                nc.vector.tensor_tensor(Oh_all[0:64, hb, :], o_pss[0][0:64, :],
                                        rb_sb[0:64, :], OP.mult)
                nc.vector.tensor_tensor(Oh_all[64:128, hb, :], o_pss[1][64:128, :],
                                        rb_sb[64:128, :], OP.mult)

            for qc in range(2):
                qs = slice(qc * 512, (qc + 1) * 512)
                # Oh_all = 64 * attn_out per head, fp8
                Oh_all = p_oh.tile([128, 4, 512], f8e4, name="Oh_all", tag="oh")
                pend = None
                for hb in range(4):
                    o_pss = []
                    for par in range(2):
                        po = par * 64
                        o_ps = pOp.tile([128, 512], f32, name="o_ps", tag="op")
                        for b2 in range(nkp // 2):
                            s2 = pS.tile([128, 2, 512], f32, name="s2", tag="s2")
                            nc.tensor.matmul(s2[0:64, 0, 0:128], ones_sq[:, :],
                                             ones_big[:, 0:128], start=True, stop=True)
                            for k2 in range(2):
                                kp = 2 * b2 + k2
                                nc.tensor.matmul(s2[:, k2, :],
                                                 K_T[po:po + 64, hb, kp * 128:(kp + 1) * 128],
                                                 Q_T[po:po + 64, hb, qs],
                                                 start=True, stop=True)
                            e2 = p_eT.tile([128, 2, 512], f8e4, name="e2", tag="et")
                            nc.scalar.activation(e2[:], s2[:], AF.Exp, scale=0.125 / 4096.0)
                            if par == 0:
                                nc.tensor.matmul(o_ps[0:65, :], V_ev[:, 2 * b2:2 * b2 + 2, hb, 0:65],
                                                 e2[:], start=(b2 == 0), stop=(b2 == nkp // 2 - 1),
                                                 perf_mode=DR)
                            else:
                                nc.tensor.matmul(o_ps[:], V_od[:, 2 * b2:2 * b2 + 2, hb, :],
                                                 e2[:], start=(b2 == 0), stop=(b2 == nkp // 2 - 1),
                                                 perf_mode=DR)
                        o_pss.append(o_ps)
                    # softmax denominators: 1/d = exp(-ln d), ln/exp table set
                    lnd = p_ms.tile([65, 512], f32, name="lnd", tag="ms")
                    nc.scalar.activation(lnd[64:65, :], o_pss[0][64:65, :], AF.Ln)
                    nc.scalar.activation(lnd[0:1, :], o_pss[1][0:1, :], AF.Ln)
                    rcb = p_ms.tile([65, 512], bf16, name="rcb", tag="ms")
                    nc.scalar.activation(rcb[64:65, :], lnd[64:65, :], AF.Exp, scale=-1.0)
                    nc.scalar.activation(rcb[0:1, :], lnd[0:1, :], AF.Exp, scale=-1.0)
                    if pend is not None:
                        emit_norm(pend)
                    pend = (o_pss, rcb, hb, Oh_all)
                emit_norm(pend)
                # o-proj: psum = (64 w)(64 attn) + 4096*bias -> /4096 + resid
                for mt in range(4):
                    ps = pOp.tile([128, 512], f32, name="ps_op", tag="op")
                    nc.tensor.matmul(ps[:], brow[0:1, (widx // 4) * 512 + mt * 128:
                                                (widx // 4) * 512 + (mt + 1) * 128],
                                     ones_q[:], start=True, stop=False)
                    for h2 in range(2):
                        nc.tensor.matmul(ps[:],
                                         w_attn[:, widx + 3, 2 * h2:2 * h2 + 2, mt * 128:(mt + 1) * 128],
                                         Oh_all[:, 2 * h2:2 * h2 + 2, :],
                                         start=False, stop=(h2 == 1), perf_mode=DR)
                    nc.vector.scalar_tensor_tensor(resid_out[:, mt, qs], ps[:],
                                                   1.0 / 4096.0,
                                                   resid_in[:, mt, qs], OP.mult, OP.add)

        def moe(xn3, X3):
            # pass 1: hid = SV * (val + bv) * silu(gate + bg), fp8 into arena
            hid_ar = sing.tile([128, 32, 1024], f8e4, name="hid_ar", tag="hid")
            for j in range(32):
                e, jj = j // 16, j % 16
                bv_i = e * 32 + jj
                bg_i = e * 32 + 16 + jj
                wv_t = p_wi.tile([128, 4, 128], f8e4, name="wv_t", tag="wi")
                dma(out=wv_t[:], in_=wi_h[bv_i, :, :].rearrange("p (t m) -> p t m", t=4))
                wg_t = p_wi.tile([128, 4, 128], f8e4, name="wg_t", tag="wi")
                dma(out=wg_t[:], in_=wi_h[bg_i, :, :].rearrange("p (t m) -> p t m", t=4))
                qsl = [slice(0, 512), slice(512, 1024)]
                gate_pss = [pOp.tile([128, 512], f32, name="gate_ps", tag="op")
                            for _ in range(2)]
                for t2 in range(2):
                    for qc in range(2):
                        nc.tensor.matmul(gate_pss[qc][:], wg_t[:, 2 * t2:2 * t2 + 2, :],
                                         xn3[:, 2 * t2:2 * t2 + 2, qsl[qc]],
                                         start=(t2 == 0), stop=(t2 == 1), perf_mode=DR)
                sgs = []
                for qc in range(2):
                    sg = p_eT.tile([128, 512], bf16, name="sg", tag="et")
                    nc.scalar.activation(sg[:], gate_pss[qc][:], AF.Silu,
                                         bias=bi_sb[:, bg_i:bg_i + 1], scale=1.0 / SI)
                    sgs.append(sg)
                val_pss = [pOp.tile([128, 512], f32, name="val_ps", tag="op")
                           for _ in range(2)]
                for t2 in range(2):
                    for qc in range(2):
                        nc.tensor.matmul(val_pss[qc][:], wv_t[:, 2 * t2:2 * t2 + 2, :],
                                         xn3[:, 2 * t2:2 * t2 + 2, qsl[qc]],
                                         start=(t2 == 0), stop=(t2 == 1), perf_mode=DR)
                for qc in range(2):
                    nc.vector.scalar_tensor_tensor(hid_ar[:, j, qsl[qc]], val_pss[qc][:],
                                                   bi_sb[:, bv_i:bv_i + 1],
                                                   sgs[qc][:], OP.add, OP.mult)
                # X3 row-major staging via DMA crossbar transpose (no PE)
                if j < 16 and j % 2 == 0:
                    tq = j // 2
                    x3r = p_x3r.tile([128, 512], bf16, name=f"x3r{tq}", tag="x3r")
                    for mt in range(4):
                        nc.sync.dma_start_transpose(
                            x3r[:, mt * 128:(mt + 1) * 128],
                            X3[:, mt, tq * 128:(tq + 1) * 128])
                    yield_tiles.append(x3r)

            # pass 2: token-major out-proj: eo[tok, d] = sum_he hid[he, tok]*wo[he, d]
            out_r = out_h.rearrange("(t p) d -> p t d", p=128)
            for qc in range(2):
                eos_a = pS.tile([128, 2, 512], f32, name="eos_a", tag="s2")
                eos_b = pS.tile([128, 2, 512], f32, name="eos_b", tag="s2")
                eslice = [eos_a[:, 0, :], eos_a[:, 1, :], eos_b[:, 0, :], eos_b[:, 1, :]]
                for tc in range(4):
                    nc.tensor.matmul(eslice[tc], borow[:, 0:128], ones_q[:, 0:512],
                                     start=True, stop=False)
                for jp in range(16):
                    wo_t = p_wo.tile([128, 2, 512], f8e4, name="wo_t", tag="wo")
                    dma(out=wo_t[:], in_=wo_h[jp, :, :].rearrange("p (t m) -> p t m", t=2))
                    for tc in range(4):
                        tok = slice(qc * 512 + tc * 128, qc * 512 + (tc + 1) * 128)
                        nc.tensor.matmul(eslice[tc],
                                         hid_ar[:, 2 * jp:2 * jp + 2, tok],
                                         wo_t[:], start=False, stop=(jp == 15),
                                         perf_mode=DR)
                for tc in range(4):
                    tq = qc * 4 + tc
                    orm = p_orm.tile([128, 512], f32, name="orm", tag="orm")
                    nc.vector.scalar_tensor_tensor(orm[:], eslice[tc], 1.0 / (SV * SO),
                                                   yield_tiles[tq][:], OP.mult, OP.add)
                    dma(out=out_r[:, tq, :], in_=orm[:])

        yield_tiles = []
        xn1 = p_xn.tile([128, 4, 1024], f8e4, name="xn1", tag="xn")
        layer_norm(X_T, xn1)
        X2 = p_resid.tile([128, 4, 1024], bf16, name="X2", tag="resid")
        attention(0, xn1, scene_T, 512, X_T, X2)
        xn2 = p_xn.tile([128, 4, 1024], f8e4, name="xn2", tag="xn")
        layer_norm(X2, xn2)
        X3 = p_resid.tile([128, 4, 1024], bf16, name="X3", tag="resid")
        attention(4, xn2, xn2, 1024, X2, X3)
        xn3 = p_xn.tile([128, 4, 1024], f8e4, name="xn3", tag="xn")
        layer_norm(X3, xn3)
        moe(xn3, X3)

    _legalize_waits(nc)
    _NC = nc
    return nc


def _legalize_waits(nc):
    # Matmult/Ldweights/DMA encodings hold a single sem wait; split extras
    # onto EventSemaphore instructions on the same queue.
    from concourse import mybir
    n = 0
    for fn in nc.m.functions:
        for blk in fn.blocks:
            out = []
            for inst in blk.instructions:
                si = getattr(inst, "sync_info", None)
                ow = list(si.on_wait) if si is not None else []
                if len(ow) > 1 and getattr(inst, "opcode", None) is not None:
                    for j, w in enumerate(ow[:-1]):
                        out.append(mybir.InstEventSemaphore(
                            name=f"{inst.name}-wx{j}",
                            engine=inst.engine,
                            sync_info=mybir.SyncInfo(on_wait=[w], on_update=[]),
                        ))
                        n += 1
                    inst.sync_info = mybir.SyncInfo(
                        on_wait=[ow[-1]], on_update=list(si.on_update))
                out.append(inst)
            blk.instructions = out
    return n


def _silu(v):
    return v / (1.0 + np.exp(-v))


def _softmax(v):
    m = v.max(axis=-1, keepdims=True)
    ex = np.exp(v - m)
    return ex / ex.sum(axis=-1, keepdims=True)


def _f8(x):
    return np.clip(x, -240.0, 240.0).astype(F8)


def _prepare(inputs):
    inp = {k: np.asarray(v, dtype=np.float32) for k, v in inputs.items()}
    x = inp["x"]
    scene = inp["scene_tokens"]
    t = inp["t"]
    g = inp["scene_norm_g"]
    bvec = inp["scene_norm_b"]

    half = D // 2
    freqs = np.exp(-math.log(10000.0) * np.arange(half, dtype=np.float32) / (half - 1)).astype(np.float32)
    ang = t[:, None] * freqs[None, :]
    temb = np.concatenate([np.cos(ang), np.sin(ang)], axis=-1).astype(np.float32)
    ncv = _silu(temb @ inp["ne_w1"] + inp["ne_b1"]) @ inp["ne_w2"] + inp["ne_b2"]

    mod1 = ncv @ inp["ncsa_mod_w"] + inp["ncsa_mod_b"]
    shift1, scale1 = mod1[:, :D], mod1[:, D:]
    mod2 = ncv @ inp["moe_mod_w"] + inp["moe_mod_b"]
    shift2, scale2 = mod2[:, :D], mod2[:, D:]

    probs = _softmax(ncv @ inp["router_w"])
    ti = np.argsort(-probs, axis=-1, kind="stable")[:, :K]
    tw = np.take_along_axis(probs, ti, axis=-1)
    tw = tw / np.clip(tw.sum(-1, keepdims=True), 1e-8, None)

    ca_wq_e = g[:, None] * inp["ca_wq"]
    ca_bq_e = inp["ca_bq"] + bvec @ inp["ca_wq"]
    ca_bo_e = inp["ca_bo"] + inp["ca_bv"] @ inp["ca_wo"]
    zero = np.zeros(D, np.float32)
    vperm = np.concatenate([np.arange(h * HD, (h + 1) * HD) for h in (0, 2, 4, 6, 1, 3, 5, 7)])

    in_maps = []
    for b in range(B):
        s1 = 1.0 + scale1[b]
        sa_wq_e = s1[:, None] * inp["sa_wq"]
        sa_bq_e = inp["sa_bq"] + shift1[b] @ inp["sa_wq"]
        sa_wk_e = s1[:, None] * inp["sa_wk"]
        sa_bk_e = inp["sa_bk"] + shift1[b] @ inp["sa_wk"]
        sa_wv_e = s1[:, None] * inp["sa_wv"]
        sa_bv_e = inp["sa_bv"] + shift1[b] @ inp["sa_wv"]
        sa_bo_e = inp["sa_bo"] + sa_bv_e @ inp["sa_wo"]

        wattn = _f8(SW * np.stack([ca_wq_e, inp["ca_wk"], inp["ca_wv"][:, vperm], inp["ca_wo"],
                                   sa_wq_e, sa_wk_e, sa_wv_e[:, vperm], inp["sa_wo"]]))
        # Q/K biases pre-scaled by SW (psum is 64x); V zero; O via brow
        battn = SW * np.stack([ca_bq_e, inp["ca_bk"], zero, zero,
                               sa_bq_e, sa_bk_e, zero, zero]).astype(np.float32)
        brow_v = np.zeros((1, 1024), np.float32)
        brow_v[0, 0:512] = SW * SW * ca_bo_e
        brow_v[0, 512:1024] = SW * SW * sa_bo_e

        s2 = 1.0 + scale2[b]
        Wis, bis, Wos = [], [], []
        bo_moe = np.zeros(D, np.float32)
        for k in range(K):
            eidx = int(ti[b, k])
            w = np.float32(tw[b, k])
            Wi_e = inp["fc_in_w"][eidx]
            Wis.append(s2[:, None] * Wi_e)
            bis.append(inp["fc_in_b"][eidx] + shift2[b] @ Wi_e)
            Wos.append(w * inp["fc_out_w"][eidx])
            bo_moe = bo_moe + w * inp["fc_out_b"][eidx]
        Wi_cat = np.concatenate(Wis, axis=1)          # [D, 2*2HE] = [512, 8192]
        bi_cat = np.concatenate(bis, axis=0)          # [8192]
        Wo_cat = np.concatenate(Wos, axis=0)          # [2*HE, D] = [4096, 512]

        # wi chunks [64][d_lo 128][d_hi 4 * h_lo 128]; val chunks x SV, gate x SI
        wi_pt = np.ascontiguousarray(
            Wi_cat.reshape(4, 128, 64, 128).transpose(2, 1, 0, 3).reshape(64, 128, 512))
        wsc = np.zeros((64, 1, 1), np.float32)
        bsc = np.zeros(64, np.float32)
        for e in range(2):
            wsc[e * 32:e * 32 + 16] = SV
            wsc[e * 32 + 16:e * 32 + 32] = SI
            bsc[e * 32:e * 32 + 16] = SV
            bsc[e * 32 + 16:e * 32 + 32] = 1.0
        wi8 = _f8(wi_pt * wsc)
        bi_pt = np.ascontiguousarray(bi_cat.reshape(64, 128) * bsc[:, None]).astype(np.float32)
        # wo chunks [16][he_lo 128][he_hi 2 * d 512], x SO
        wo8 = _f8(SO * np.ascontiguousarray(
            Wo_cat.reshape(16, 2, 128, 512).transpose(0, 2, 1, 3).reshape(16, 128, 1024)))
        borow_v = (SV * SO * bo_moe).reshape(1, 512)

        in_maps.append({
            "x": np.ascontiguousarray(x[b]).astype(BF16),
            "scene": np.ascontiguousarray(scene[b]).astype(BF16),
            "wattn": np.ascontiguousarray(wattn),
            "battn": np.ascontiguousarray(battn),
            "brow": brow_v.astype(BF16),
            "wi": wi8,
            "bi_t": bi_pt,
            "wo": wo8,
            "borow": borow_v.astype(BF16),
        })
    return in_maps


def _run(in_maps, trace=False):
    from concourse.bass_utils import run_bass_kernel_spmd
    nc = _build()
    return run_bass_kernel_spmd(nc, in_maps, list(range(NCORES)), trace=trace)


def kernel(**inputs):
    in_maps = _prepare(inputs)
    res = _run(in_maps)
    return np.stack([np.asarray(res.results[i]["out"], dtype=np.float32) for i in range(B)])


# revision 14
# speedup vs baseline: 1.8977x; 1.0240x over previous
import math
import sys

for _p in ("/root/.axon_site", "/root/.axon_site/_ro/trn_rl_repo", "/opt/trn_rl_repo"):
    if _p not in sys.path:
        sys.path.append(_p)

import numpy as np
import ml_dtypes

BF16 = ml_dtypes.bfloat16
F8 = ml_dtypes.float8_e4m3  # IEEE-style e4m3: max 240 == TRN FP8_EXP4

B, L, NS = 8, 1024, 512
D, NH, DN = 512, 8, 256
E, K, HE = 8, 2, 2048
HD = D // NH
EPS = 1e-5
NCORES = 8

SW = 64.0    # attention weight fp8 scale
SI = 64.0    # moe gate fc_in scale
SV = 32.0    # moe val fc_in scale
SO = 64.0    # moe fc_out scale

_NC = None


def _build():
    global _NC
    if _NC is not None:
        return _NC
    from concourse import bass, tile, mybir, masks

    f32 = mybir.dt.float32
    bf16 = mybir.dt.bfloat16
    f8e4 = mybir.dt.float8e4
    AF = mybir.ActivationFunctionType
    OP = mybir.AluOpType
    DR = mybir.MatmulPerfMode.DoubleRow

    nc = bass.Bass()
    x_h = nc.declare_dram_parameter("x", [L, D], bf16, isOutput=False)
    scene_h = nc.declare_dram_parameter("scene", [NS, D], bf16, isOutput=False)
    wattn_h = nc.declare_dram_parameter("wattn", [8, D, D], f8e4, isOutput=False)
    battn_h = nc.declare_dram_parameter("battn", [8, D], f32, isOutput=False)
    brow_h = nc.declare_dram_parameter("brow", [1, 1024], bf16, isOutput=False)
    wi_h = nc.declare_dram_parameter("wi", [64, 128, 512], f8e4, isOutput=False)
    bi_h = nc.declare_dram_parameter("bi_t", [64, 128], f32, isOutput=False)
    wo_h = nc.declare_dram_parameter("wo", [16, 128, 1024], f8e4, isOutput=False)
    borow_h = nc.declare_dram_parameter("borow", [1, 512], bf16, isOutput=False)
    out_h = nc.declare_dram_parameter("out", [L, D], f32, isOutput=True)

    with tile.TileContext(nc) as tc, \
         tc.tile_pool(name="sing", bufs=1) as sing, \
         tc.tile_pool(name="p_rm", bufs=2) as p_rm, \
         tc.tile_pool(name="p_resid", bufs=2) as p_resid, \
         tc.tile_pool(name="p_xn", bufs=2) as p_xn, \
         tc.tile_pool(name="p_qkv", bufs=1) as p_qkv, \
         tc.tile_pool(name="p_oh", bufs=2) as p_oh, \
         tc.tile_pool(name="p_eT", bufs=4) as p_eT, \
         tc.tile_pool(name="p_ms", bufs=8) as p_ms, \
         tc.tile_pool(name="p_msx", bufs=4) as p_msx, \
         tc.tile_pool(name="p_xb", bufs=2) as p_xb, \
         tc.tile_pool(name="p_wi", bufs=6) as p_wi, \
         tc.tile_pool(name="p_wo", bufs=2) as p_wo, \
         tc.tile_pool(name="p_x3r", bufs=8) as p_x3r, \
         tc.tile_pool(name="p_orm", bufs=2) as p_orm, \
         tc.tile_pool(name="p_nt", bufs=2) as p_nt, \
         tc.tile_pool(name="pS", bufs=2, space="PSUM") as pS, \
         tc.tile_pool(name="pOp", bufs=3, space="PSUM") as pOp, \
         tc.tile_pool(name="pB", bufs=1, space="PSUM") as pB:

        dma = nc.sync.dma_start

        ident = sing.tile([128, 128], f32, name="ident", tag="ident")
        masks.make_identity(nc, ident[:])
        ones_bf = sing.tile([128, 1], bf16, name="ones_bf", tag="ones")
        nc.vector.memset(ones_bf[:], 1.0)
        ones_sq = sing.tile([128, 64], bf16, name="ones_sq", tag="ones_sq")
        nc.vector.memset(ones_sq[:], 1.0)
        ones_q = sing.tile([1, 512], bf16, name="ones_q", tag="ones_qq")
        nc.vector.memset(ones_q[:], 1.0)
        ones_row = sing.tile([1, 128], bf16, name="ones_row", tag="ones_r")
        nc.vector.memset(ones_row[:], 1.0)
        ones_big = sing.tile([128, 512], bf16, name="ones_big", tag="ones_b")
        nc.vector.memset(ones_big[:], 1.0)

        # HAM warm-up spins + ln/exp table preload during the DMA window
        dummy = p_ms.tile([1, 1], f32, name="dummy", tag="ms")
        nc.scalar.activation(dummy[:], ident[0:1, 0:1], AF.Ln)
        dummy2 = p_ms.tile([1, 1], f32, name="dummy2", tag="ms")
        nc.scalar.activation(dummy2[:], ident[0:1, 0:1], AF.Exp)
        for _w in range(28):
            spin = pS.tile([128, 2, 512], f32, name="spin", tag="s2")
            nc.tensor.matmul(spin[0:64, 0, :], ones_sq[:, :], ones_big[:],
                             start=True, stop=True)

        # x -> feature-major bf16 spine directly via DMA crossbar transpose
        X_T = p_resid.tile([128, 4, 1024], bf16, name="X_T", tag="resid")
        for mt in range(4):
            nc.sync.dma_start_transpose(X_T[:, mt, :],
                                        x_h[:, mt * 128:(mt + 1) * 128])
        scene_Tb = sing.tile([128, 4, 512], bf16, name="scene_Tb", tag="scTb")
        for mt in range(4):
            nc.sync.dma_start_transpose(scene_Tb[:, mt, :],
                                        scene_h[:, mt * 128:(mt + 1) * 128])
        w_attn = sing.tile([128, 8, 4, 512], f8e4, name="w_attn", tag="w_attn")
        dma(out=w_attn[:], in_=wattn_h.rearrange("i (t p) d -> p i t d", p=128))
        b_attn = sing.tile([128, 8, 4], f32, name="b_attn", tag="b_attn")
        dma(out=b_attn[:], in_=battn_h.rearrange("i (t p) -> p i t", p=128))
        brow = sing.tile([1, 1024], bf16, name="brow", tag="brow")
        dma(out=brow[:], in_=brow_h[:, :])
        borow = sing.tile([1, 512], bf16, name="borow", tag="borow")
        dma(out=borow[:], in_=borow_h[:, :])
        bi_sb = sing.tile([128, 64], f32, name="bi_sb", tag="bi_sb")
        dma(out=bi_sb[:], in_=bi_h.rearrange("b p -> p b"))

        scene_T = sing.tile([128, 4, 512], f8e4, name="scene_T", tag="scene_T")
        nc.vector.tensor_scalar_mul(scene_T[:], scene_Tb[:], 1.0)

        def layer_norm(src, xn):
            for qc in range(2):
                qs = slice(qc * 512, (qc + 1) * 512)
                sq = p_xb.tile([128, 4, 512], bf16, name="sq", tag="xbsq")
                nc.scalar.activation(sq[:], src[:, :, qs], AF.Square)
                sum_ps = pOp.tile([1, 512], f32, name="sum_ps", tag="op")
                for kt in range(4):
                    nc.tensor.matmul(sum_ps[:], ones_bf[:], src[:, kt, qs],
                                     start=(kt == 0), stop=(kt == 3))
                sq_ps = pOp.tile([1, 512], f32, name="sq_ps", tag="op")
                for kt in range(4):
                    nc.tensor.matmul(sq_ps[:], ones_bf[:], sq[:, kt, :],
                                     start=(kt == 0), stop=(kt == 3))
                m_bf = p_ms.tile([1, 512], bf16, name="m_bf", tag="ms")
                nc.vector.tensor_scalar_mul(m_bf[:], sum_ps[:], 1.0 / 512.0)
                m = p_ms.tile([1, 512], f32, name="m", tag="ms")
                nc.vector.tensor_scalar_mul(m[:], sum_ps[:], 1.0 / 512.0)
                e2 = p_ms.tile([1, 512], f32, name="e2", tag="ms")
                nc.vector.tensor_scalar(e2[:], sq_ps[:], 1.0 / 512.0, EPS, OP.mult, OP.add)
                mm = p_ms.tile([1, 512], f32, name="mm", tag="ms")
                nc.vector.tensor_tensor(mm[:], m[:], m[:], OP.mult)
                var = p_ms.tile([1, 512], f32, name="var", tag="ms")
                nc.vector.tensor_tensor(var[:], e2[:], mm[:], OP.subtract)
                # 1/sqrt(var) = exp(-0.5 ln var): stays in the ln/exp table set
                lnv = p_ms.tile([1, 512], f32, name="lnv", tag="ms")
                nc.scalar.activation(lnv[:], var[:], AF.Ln)
                r_bf = p_ms.tile([1, 512], bf16, name="r_bf", tag="ms")
                nc.scalar.activation(r_bf[:], lnv[:], AF.Exp, scale=-0.5)
                rb_ps = pB.tile([128, 512], f32, name="rb_ps", tag="pb")
                nc.tensor.matmul(rb_ps[:], ones_row[:], r_bf[:], start=True, stop=True)
                mb_ps = pOp.tile([128, 512], f32, name="mb_ps", tag="op")
                nc.tensor.matmul(mb_ps[:], ones_row[:], m_bf[:], start=True, stop=True)
                for mt in range(4):
                    xs = p_msx.tile([128, 512], bf16, name="xs", tag="msx")
                    nc.vector.tensor_tensor(xs[:], src[:, mt, qs], mb_ps[:], OP.subtract)
                    nc.vector.tensor_tensor(xn[:, mt, qs], xs[:], rb_ps[:], OP.mult)

        def attention(widx, xq_T, kv_T, kv_len, resid_in, resid_out):
            nkp = kv_len // 128
            nkc = kv_len // 512
            # fp8 DoubleRow projections: psum = 64*W @ x
            Q_T = p_qkv.tile([128, 4, 1024], bf16, name="Q_T", tag="q")
            for mt in range(4):
                q_pss = [pOp.tile([128, 512], f32, name="q_ps", tag="op")
                         for _ in range(2)]
                for t2 in range(2):
                    for qc in range(2):
                        nc.tensor.matmul(q_pss[qc][:],
                                         w_attn[:, widx, 2 * t2:2 * t2 + 2, mt * 128:(mt + 1) * 128],
                                         xq_T[:, 2 * t2:2 * t2 + 2, qc * 512:(qc + 1) * 512],
                                         start=(t2 == 0), stop=(t2 == 1), perf_mode=DR)
                for qc in range(2):
                    nc.vector.tensor_scalar_add(Q_T[:, mt, qc * 512:(qc + 1) * 512],
                                                q_pss[qc][:],
                                                b_attn[:, widx, mt:mt + 1])
            K_T = p_qkv.tile([128, 4, 1024], bf16, name="K_T", tag="k")
            for mt in range(4):
                k_pss = [pOp.tile([128, 512], f32, name="k_ps", tag="op")
                         for _ in range(nkc)]
                for t2 in range(2):
                    for kc in range(nkc):
                        nc.tensor.matmul(k_pss[kc][:],
                                         w_attn[:, widx + 1, 2 * t2:2 * t2 + 2, mt * 128:(mt + 1) * 128],
                                         kv_T[:, 2 * t2:2 * t2 + 2, kc * 512:(kc + 1) * 512],
                                         start=(t2 == 0), stop=(t2 == 1), perf_mode=DR)
                for kc in range(nkc):
                    nc.vector.tensor_scalar_add(K_T[:, mt, kc * 512:(kc + 1) * 512],
                                                k_pss[kc][:],
                                                b_attn[:, widx + 1, mt:mt + 1])
            # V in fp8 (64x), denominator ones packed; pad 65->68 for DR stride
            V_ev = p_qkv.tile([128, 8, 4, 68], f8e4, name="V_ev", tag="ve")
            V_od = p_qkv.tile([128, 8, 4, 128], f8e4, name="V_od", tag="vo")
            nc.vector.memset(V_ev[:, 0:nkp, :, 64:65], 1.0)
            nc.vector.memset(V_od[:, 0:nkp, :, 0:1], 1.0)
            nc.vector.memset(V_od[:, 0:nkp, :, 1:64], 0.0)
            for kp in range(nkp):
                v_ps = pOp.tile([128, 2, 4, 64], f32, name="v_ps", tag="op")
                for t2 in range(2):
                    nc.tensor.matmul(v_ps[:],
                                     kv_T[:, 2 * t2:2 * t2 + 2, kp * 128:(kp + 1) * 128],
                                     w_attn[:, widx + 2, 2 * t2:2 * t2 + 2, :],
                                     start=(t2 == 0), stop=(t2 == 1), perf_mode=DR)
                nc.vector.tensor_scalar_mul(V_ev[:, kp, :, 0:64], v_ps[:, 0, :, :], 1.0)
                nc.vector.tensor_scalar_mul(V_od[:, kp, :, 64:128], v_ps[:, 1, :, :], 1.0)

            def emit_norm(st):
                o_pss, rcb, hb, Oh_all = st
                rb_ps = pB.tile([128, 512], f32, name="rb_ps", tag="pb")
                nc.tensor.matmul(rb_ps[0:64, :], ones_sq[64:65, 0:64], rcb[64:65, :],
                                 start=True, stop=True)
                nc.tensor.matmul(rb_ps[64:128, :], ones_sq[0:1, 0:64], rcb[0:1, :],
                                 start=True, stop=True)
                den_sb = p_nt.tile([128, 512], f32, name="den_sb", tag="dsb")
                nc.vector.tensor_scalar_mul(den_sb[:], rb_ps[:], 1.0)
                z0 = 1.0 / float(kv_len)
                y1 = p_nt.tile([128, 512], f32, name="y1", tag="y1")
                nc.gpsimd.tensor_scalar(y1[:], den_sb[:], -z0 * z0, 2.0 * z0,
                                        OP.mult, OP.add)
                tn = p_nt.tile([128, 512], f32, name="tn", tag="tn")
                nc.gpsimd.tensor_tensor(tn[:], den_sb[:], y1[:], OP.mult)
                un = p_nt.tile([128, 512], f32, name="un", tag="un")
                nc.gpsimd.tensor_scalar(un[:], tn[:], -1.0, 2.0, OP.mult, OP.add)
                rb_sb = p_eT.tile([128, 512], bf16, name="rb_sb", tag="rbs")
                nc.gpsimd.tensor_tensor(rb_sb[:], y1[:], un[:], OP.mult)
                nc.vector.tensor_tensor(Oh_all[0:64, hb, :], o_pss[0][0:64, :],
                                        rb_sb[0:64, :], OP.mult)
                nc.vector.tensor_tensor(Oh_all[64:128, hb, :], o_pss[1][64:128, :],
                                        rb_sb[64:128, :], OP.mult)

            for qc in range(2):
                qs = slice(qc * 512, (qc + 1) * 512)
                # Oh_all = 64 * attn_out per head, fp8
                Oh_all = p_oh.tile([128, 4, 512], f8e4, name="Oh_all", tag="oh")
                pend = None
                for hb in range(4):
                    o_pss = []
                    for par in range(2):
                        po = par * 64
                        o_ps = pOp.tile([128, 512], f32, name="o_ps", tag="op")
                        for b2 in range(nkp // 2):
                            s2 = pS.tile([128, 2, 512], f32, name="s2", tag="s2")
                            for k2 in range(2):
                                kp = 2 * b2 + k2
                                nc.tensor.matmul(s2[:, k2, :],
                                                 K_T[po:po + 64, hb, kp * 128:(kp + 1) * 128],
                                                 Q_T[po:po + 64, hb, qs],
                                                 start=True, stop=True)
                            e2 = p_eT.tile([128, 2, 512], f8e4, name="e2", tag="et")
                            nc.scalar.activation(e2[:], s2[:], AF.Exp, scale=0.125 / 4096.0)
                            if par == 0:
                                nc.tensor.matmul(o_ps[0:65, :], V_ev[:, 2 * b2:2 * b2 + 2, hb, 0:65],
                                                 e2[:], start=(b2 == 0), stop=(b2 == nkp // 2 - 1),
                                                 perf_mode=DR)
                            else:
                                nc.tensor.matmul(o_ps[:], V_od[:, 2 * b2:2 * b2 + 2, hb, :],
                                                 e2[:], start=(b2 == 0), stop=(b2 == nkp // 2 - 1),
                                                 perf_mode=DR)
                        o_pss.append(o_ps)
                    # softmax denominator rows -> SBUF bf16 (for the PE
                    # broadcast; 1/d is computed by GpSimd Newton off both
                    # hot engines)
                    rcb = p_ms.tile([65, 512], bf16, name="rcb", tag="ms")
                    nc.vector.tensor_scalar_mul(rcb[64:65, :], o_pss[0][64:65, :], 1.0)
                    nc.vector.tensor_scalar_mul(rcb[0:1, :], o_pss[1][0:1, :], 1.0)
                    if pend is not None:
                        emit_norm(pend)
                    pend = (o_pss, rcb, hb, Oh_all)
                emit_norm(pend)
                # o-proj: psum = (64 w)(64 attn) + 4096*bias -> /4096 + resid
                for mt in range(4):
                    ps = pOp.tile([128, 512], f32, name="ps_op", tag="op")
                    nc.tensor.matmul(ps[:], brow[0:1, (widx // 4) * 512 + mt * 128:
                                                (widx // 4) * 512 + (mt + 1) * 128],
                                     ones_q[:], start=True, stop=False)
                    for h2 in range(2):
                        nc.tensor.matmul(ps[:],
                                         w_attn[:, widx + 3, 2 * h2:2 * h2 + 2, mt * 128:(mt + 1) * 128],
                                         Oh_all[:, 2 * h2:2 * h2 + 2, :],
                                         start=False, stop=(h2 == 1), perf_mode=DR)
                    nc.vector.scalar_tensor_tensor(resid_out[:, mt, qs], ps[:],
                                                   1.0 / 4096.0,
                                                   resid_in[:, mt, qs], OP.mult, OP.add)

        def moe(xn3, X3):
            # pass 1: hid = SV * (val + bv) * silu(gate + bg), fp8 into arena
            hid_ar = sing.tile([128, 32, 1024], f8e4, name="hid_ar", tag="hid")
            for j in range(32):
                e, jj = j // 16, j % 16
                bv_i = e * 32 + jj
                bg_i = e * 32 + 16 + jj
                wv_t = p_wi.tile([128, 4, 128], f8e4, name="wv_t", tag="wi")
                dma(out=wv_t[:], in_=wi_h[bv_i, :, :].rearrange("p (t m) -> p t m", t=4))
                wg_t = p_wi.tile([128, 4, 128], f8e4, name="wg_t", tag="wi")
                dma(out=wg_t[:], in_=wi_h[bg_i, :, :].rearrange("p (t m) -> p t m", t=4))
                qsl = [slice(0, 512), slice(512, 1024)]
                gate_pss = [pOp.tile([128, 512], f32, name="gate_ps", tag="op")
                            for _ in range(2)]
                for t2 in range(2):
                    for qc in range(2):
                        nc.tensor.matmul(gate_pss[qc][:], wg_t[:, 2 * t2:2 * t2 + 2, :],
                                         xn3[:, 2 * t2:2 * t2 + 2, qsl[qc]],
                                         start=(t2 == 0), stop=(t2 == 1), perf_mode=DR)
                sgs = []
                for qc in range(2):
                    sg = p_eT.tile([128, 512], bf16, name="sg", tag="et")
                    nc.scalar.activation(sg[:], gate_pss[qc][:], AF.Silu,
                                         bias=bi_sb[:, bg_i:bg_i + 1], scale=1.0 / SI)
                    sgs.append(sg)
                val_pss = [pOp.tile([128, 512], f32, name="val_ps", tag="op")
                           for _ in range(2)]
                for t2 in range(2):
                    for qc in range(2):
                        nc.tensor.matmul(val_pss[qc][:], wv_t[:, 2 * t2:2 * t2 + 2, :],
                                         xn3[:, 2 * t2:2 * t2 + 2, qsl[qc]],
                                         start=(t2 == 0), stop=(t2 == 1), perf_mode=DR)
                for qc in range(2):
                    nc.vector.scalar_tensor_tensor(hid_ar[:, j, qsl[qc]], val_pss[qc][:],
                                                   bi_sb[:, bv_i:bv_i + 1],
                                                   sgs[qc][:], OP.add, OP.mult)
                # X3 row-major staging via DMA crossbar transpose (no PE)
                if j < 16 and j % 2 == 0:
                    tq = j // 2
                    x3r = p_x3r.tile([128, 512], bf16, name=f"x3r{tq}", tag="x3r")
                    for mt in range(4):
                        nc.sync.dma_start_transpose(
                            x3r[:, mt * 128:(mt + 1) * 128],
                            X3[:, mt, tq * 128:(tq + 1) * 128])
                    yield_tiles.append(x3r)

            # pass 2: token-major out-proj: eo[tok, d] = sum_he hid[he, tok]*wo[he, d]
            out_r = out_h.rearrange("(t p) d -> p t d", p=128)
            for qc in range(2):
                eos_a = pS.tile([128, 2, 512], f32, name="eos_a", tag="s2")
                eos_b = pS.tile([128, 2, 512], f32, name="eos_b", tag="s2")
                eslice = [eos_a[:, 0, :], eos_a[:, 1, :], eos_b[:, 0, :], eos_b[:, 1, :]]
                for tc in range(4):
                    nc.tensor.matmul(eslice[tc], borow[:, 0:128], ones_q[:, 0:512],
                                     start=True, stop=False)
                for jp in range(16):
                    wo_t = p_wo.tile([128, 2, 512], f8e4, name="wo_t", tag="wo")
                    dma(out=wo_t[:], in_=wo_h[jp, :, :].rearrange("p (t m) -> p t m", t=2))
                    for tc in range(4):
                        tok = slice(qc * 512 + tc * 128, qc * 512 + (tc + 1) * 128)
                        nc.tensor.matmul(eslice[tc],
                                         hid_ar[:, 2 * jp:2 * jp + 2, tok],
                                         wo_t[:], start=False, stop=(jp == 15),
                                         perf_mode=DR)
                for tc in range(4):
                    tq = qc * 4 + tc
                    orm = p_orm.tile([128, 512], f32, name="orm", tag="orm")
                    nc.vector.scalar_tensor_tensor(orm[:], eslice[tc], 1.0 / (SV * SO),
                                                   yield_tiles[tq][:], OP.mult, OP.add)
                    dma(out=out_r[:, tq, :], in_=orm[:])

        yield_tiles = []
        xn1 = p_xn.tile([128, 4, 1024], f8e4, name="xn1", tag="xn")
        layer_norm(X_T, xn1)
        X2 = p_resid.tile([128, 4, 1024], bf16, name="X2", tag="resid")
        attention(0, xn1, scene_T, 512, X_T, X2)
        xn2 = p_xn.tile([128, 4, 1024], f8e4, name="xn2", tag="xn")
        layer_norm(X2, xn2)
        X3 = p_resid.tile([128, 4, 1024], bf16, name="X3", tag="resid")
        attention(4, xn2, xn2, 1024, X2, X3)
        xn3 = p_xn.tile([128, 4, 1024], f8e4, name="xn3", tag="xn")
        layer_norm(X3, xn3)
        moe(xn3, X3)

    _legalize_waits(nc)
    _NC = nc
    return nc


def _legalize_waits(nc):
    # Matmult/Ldweights/DMA encodings hold a single sem wait; split extras
    # onto EventSemaphore instructions on the same queue.
    from concourse import mybir
    n = 0
    for fn in nc.m.functions:
        for blk in fn.blocks:
            out = []
            for inst in blk.instructions:
                si = getattr(inst, "sync_info", None)
                ow = list(si.on_wait) if si is not None else []
                if len(ow) > 1 and getattr(inst, "opcode", None) is not None:
                    for j, w in enumerate(ow[:-1]):
                        out.append(mybir.InstEventSemaphore(
                            name=f"{inst.name}-wx{j}",
                            engine=inst.engine,
                            sync_info=mybir.SyncInfo(on_wait=[w], on_update=[]),
                        ))
                        n += 1
                    inst.sync_info = mybir.SyncInfo(
                        on_wait=[ow[-1]], on_update=list(si.on_update))
                out.append(inst)
            blk.instructions = out
    return n


def _silu(v):
    return v / (1.0 + np.exp(-v))


def _softmax(v):
    m = v.max(axis=-1, keepdims=True)
    ex = np.exp(v - m)
    return ex / ex.sum(axis=-1, keepdims=True)


def _f8(x):
    return np.clip(x, -240.0, 240.0).astype(F8)


def _prepare(inputs):
    inp = {k: np.asarray(v, dtype=np.float32) for k, v in inputs.items()}
    x = inp["x"]
    scene = inp["scene_tokens"]
    t = inp["t"]
    g = inp["scene_norm_g"]
    bvec = inp["scene_norm_b"]

    half = D // 2
    freqs = np.exp(-math.log(10000.0) * np.arange(half, dtype=np.float32) / (half - 1)).astype(np.float32)
    ang = t[:, None] * freqs[None, :]
    temb = np.concatenate([np.cos(ang), np.sin(ang)], axis=-1).astype(np.float32)
    ncv = _silu(temb @ inp["ne_w1"] + inp["ne_b1"]) @ inp["ne_w2"] + inp["ne_b2"]

    mod1 = ncv @ inp["ncsa_mod_w"] + inp["ncsa_mod_b"]
    shift1, scale1 = mod1[:, :D], mod1[:, D:]
    mod2 = ncv @ inp["moe_mod_w"] + inp["moe_mod_b"]
    shift2, scale2 = mod2[:, :D], mod2[:, D:]

    probs = _softmax(ncv @ inp["router_w"])
    ti = np.argsort(-probs, axis=-1, kind="stable")[:, :K]
    tw = np.take_along_axis(probs, ti, axis=-1)
    tw = tw / np.clip(tw.sum(-1, keepdims=True), 1e-8, None)

    ca_wq_e = g[:, None] * inp["ca_wq"]
    ca_bq_e = inp["ca_bq"] + bvec @ inp["ca_wq"]
    ca_bo_e = inp["ca_bo"] + inp["ca_bv"] @ inp["ca_wo"]
    zero = np.zeros(D, np.float32)
    vperm = np.concatenate([np.arange(h * HD, (h + 1) * HD) for h in (0, 2, 4, 6, 1, 3, 5, 7)])

    in_maps = []
    for b in range(B):
        s1 = 1.0 + scale1[b]
        sa_wq_e = s1[:, None] * inp["sa_wq"]
        sa_bq_e = inp["sa_bq"] + shift1[b] @ inp["sa_wq"]
        sa_wk_e = s1[:, None] * inp["sa_wk"]
        sa_bk_e = inp["sa_bk"] + shift1[b] @ inp["sa_wk"]
        sa_wv_e = s1[:, None] * inp["sa_wv"]
        sa_bv_e = inp["sa_bv"] + shift1[b] @ inp["sa_wv"]
        sa_bo_e = inp["sa_bo"] + sa_bv_e @ inp["sa_wo"]

        wattn = _f8(SW * np.stack([ca_wq_e, inp["ca_wk"], inp["ca_wv"][:, vperm], inp["ca_wo"],
                                   sa_wq_e, sa_wk_e, sa_wv_e[:, vperm], inp["sa_wo"]]))
        # Q/K biases pre-scaled by SW (psum is 64x); V zero; O via brow
        battn = SW * np.stack([ca_bq_e, inp["ca_bk"], zero, zero,
                               sa_bq_e, sa_bk_e, zero, zero]).astype(np.float32)
        brow_v = np.zeros((1, 1024), np.float32)
        brow_v[0, 0:512] = SW * SW * ca_bo_e
        brow_v[0, 512:1024] = SW * SW * sa_bo_e

        s2 = 1.0 + scale2[b]
        Wis, bis, Wos = [], [], []
        bo_moe = np.zeros(D, np.float32)
        for k in range(K):
            eidx = int(ti[b, k])
            w = np.float32(tw[b, k])
            Wi_e = inp["fc_in_w"][eidx]
            Wis.append(s2[:, None] * Wi_e)
            bis.append(inp["fc_in_b"][eidx] + shift2[b] @ Wi_e)
            Wos.append(w * inp["fc_out_w"][eidx])
            bo_moe = bo_moe + w * inp["fc_out_b"][eidx]
        Wi_cat = np.concatenate(Wis, axis=1)          # [D, 2*2HE] = [512, 8192]
        bi_cat = np.concatenate(bis, axis=0)          # [8192]
        Wo_cat = np.concatenate(Wos, axis=0)          # [2*HE, D] = [4096, 512]

        # wi chunks [64][d_lo 128][d_hi 4 * h_lo 128]; val chunks x SV, gate x SI
        wi_pt = np.ascontiguousarray(
            Wi_cat.reshape(4, 128, 64, 128).transpose(2, 1, 0, 3).reshape(64, 128, 512))
        wsc = np.zeros((64, 1, 1), np.float32)
        bsc = np.zeros(64, np.float32)
        for e in range(2):
            wsc[e * 32:e * 32 + 16] = SV
            wsc[e * 32 + 16:e * 32 + 32] = SI
            bsc[e * 32:e * 32 + 16] = SV
            bsc[e * 32 + 16:e * 32 + 32] = 1.0
        wi8 = _f8(wi_pt * wsc)
        bi_pt = np.ascontiguousarray(bi_cat.reshape(64, 128) * bsc[:, None]).astype(np.float32)
        # wo chunks [16][he_lo 128][he_hi 2 * d 512], x SO
        wo8 = _f8(SO * np.ascontiguousarray(
            Wo_cat.reshape(16, 2, 128, 512).transpose(0, 2, 1, 3).reshape(16, 128, 1024)))
        borow_v = (SV * SO * bo_moe).reshape(1, 512)

        in_maps.append({
            "x": np.ascontiguousarray(x[b]).astype(BF16),
            "scene": np.ascontiguousarray(scene[b]).astype(BF16),
            "wattn": np.ascontiguousarray(wattn),
            "battn": np.ascontiguousarray(battn),
            "brow": brow_v.astype(BF16),
            "wi": wi8,
            "bi_t": bi_pt,
            "wo": wo8,
            "borow": borow_v.astype(BF16),
        })
    return in_maps


def _run(in_maps, trace=False):
    from concourse.bass_utils import run_bass_kernel_spmd
    nc = _build()
    return run_bass_kernel_spmd(nc, in_maps, list(range(NCORES)), trace=trace)


def kernel(**inputs):
    in_maps = _prepare(inputs)
    res = _run(in_maps)
    return np.stack([np.asarray(res.results[i]["out"], dtype=np.float32) for i in range(B)])
